# revision 7
# baseline (speedup 1.0000x reference)
"""MetaNCA fused kernel for 8 TRN2 NeuronCores.

Sharding: the [in_units, out_units] pair grid is sharded along out_units (j)
across the 8 cores — each core owns a 384-column block of the weight matrix,
computes its block of the pairwise-MLP updates, the (local-column) bias
update, X @ new_w[:, block] and a j-sharded softmax.  Cross-core traffic is
one AllReduce of the row_part partial ([10, 3072], contracted over local j)
and one AllGather of per-core softmax stats ([2048, 2]).

Pair-MLP mapping: groups of 12 i-rows are packed on the partition axis as
(m, h) = 12 rows x 10 hidden units = 120 partitions.  Stage 1 (relu of
row_i + col_j + b1) is a single DVE tensor_scalar per group; stages 2/3 are
PE matmuls with a block-diagonal W2^T and a shifted block-diagonal W3 that
accumulates straight into a PSUM bank pre-filled (by an identity matmul)
with the corresponding weight rows, so new_w = weight + updates materializes
in PSUM for free.
"""

import numpy as np

import concourse.bass as bass
import concourse.mybir as mybir
import concourse.tile as tile
from concourse import bacc
from concourse import bass_utils
from concourse.masks import make_identity

F32 = mybir.dt.float32
AX = mybir.AxisListType
OP = mybir.AluOpType
ACT = mybir.ActivationFunctionType

IN, OUT, B, H = 3072, 3072, 2048, 10
NCORES = 8
JL = OUT // NCORES          # 384 local out_units per core
GRP = 12                    # i rows packed per group (12*10 = 120 partitions)
TIL = 120                   # i rows per psum/new_w tile (10 groups)
TILE_ROWS = [TIL] * 25 + [IN - 25 * TIL]   # 25 x 120 + 72
NT = len(TILE_ROWS)         # 26
NBT = B // 128              # 16 b-tiles
RG = [list(range(NCORES))]

TRACE = False               # set by test.py to capture an NTFF trace
LAST_RESULTS = None


def _build_body(nc, tc, d):
    """Emit the whole per-core program. d: dict of dram tensor handles."""
    ctx_pools = []

    def pool(name, **kw):
        cm = tc.tile_pool(name=name, **kw)
        p = cm.__enter__()
        ctx_pools.append(cm)
        return p

    p_const = pool("const", bufs=1)
    p_w = pool("wcols", bufs=1)
    p_nw = pool("neww", bufs=1)
    p_e = pool("etiles", bufs=1)
    p_work = pool("work", bufs=1)
    p_dram = pool("dram", bufs=1, space="DRAM")

    sync = nc.sync

    # ---------------- constants / small inputs -> SBUF ----------------
    w2t_sb = p_const.tile([H, H], F32)
    w3c_sb = p_const.tile([H, 1], F32)
    w1c_sb = p_const.tile([H, 1], F32)
    b1c_sb = p_const.tile([H, 1], F32)
    b2c_sb = p_const.tile([H, 1], F32)
    b2t_sb = p_const.tile([TIL, 1], F32)
    b3b_sb = p_const.tile([128, 1], F32)
    biasj_sb = p_const.tile([1, JL], F32)
    for t, name in [(w2t_sb, "w2t"), (w3c_sb, "w3c"), (w1c_sb, "w1c"),
                    (b1c_sb, "b1c"), (b2c_sb, "b2c"), (b2t_sb, "b2t"),
                    (b3b_sb, "b3b"), (biasj_sb, "biasj")]:
        sync.dma_start(t[:], d[name].ap())

    ones10 = p_const.tile([1, H], F32)
    nc.vector.memset(ones10[:], 1.0)
    ones128 = p_const.tile([1, 128], F32)
    nc.vector.memset(ones128[:], 1.0)
    ident = p_const.tile([128, 128], F32)
    make_identity(nc, ident[:])

    # block-diag W2^T [120,120]: block m rows/cols [10m:10m+10] = W2^T
    bdw2t = p_const.tile([TIL, TIL], F32)
    nc.vector.memset(bdw2t[:], 0.0)
    # shifted block-diag W3 [120, 240]: bw3[10m+g, 108+m] = W3[0, g]
    bw3 = p_const.tile([TIL, 240], F32)
    nc.vector.memset(bw3[:], 0.0)
    for m in range(GRP):
        sync.dma_start(bdw2t[10 * m:10 * m + 10, 10 * m:10 * m + 10], w2t_sb[:])
        sync.dma_start(bw3[10 * m:10 * m + 10, 108 + m:109 + m], w3c_sb[:])

    # ---------------- weight tiles ----------------
    w1bt_t = []
    wc_t = []
    for t in range(NT):
        i0, rows = TIL * t, TILE_ROWS[t]
        wct = p_w.tile([rows, JL], F32, tag=f"wc{t}", name=f"wc{t}")
        sync.dma_start(wct[:], d["wcols"].ap()[i0:i0 + rows, :])
        wc_t.append(wct)
        w1btt = p_const.tile([rows, H], F32, tag=f"w1bt{t}", name=f"w1bt{t}")
        sync.dma_start(w1btt[:], d["w1bt"].ap()[i0:i0 + rows, :])
        w1bt_t.append(w1btt)

    # ---------------- phase 1: partials + tiny prologue math ----------------
    aT_sb = p_const.tile([H, IN], F32)        # row_part^T after AllReduce
    colbT = p_const.tile([H, JL], F32)        # col_part^T + bias*W1c + b1
    tmp_sb = p_const.tile([H, JL], F32)       # bias*W1c + b1  (reused for new_b)
    cb = p_const.tile([TIL, JL], F32)         # colbT replicated 12x
    a_pk = p_const.tile([TIL, IN // GRP], F32)  # a packed [120, 256]

    with tc.tile_pool(name="ppearly", space="PSUM", bufs=1) as pp, \
         tc.tile_pool(name="ph1", bufs=1) as p_ph1:
        wt_t = []
        w1at_t = []
        for c in range(3):
            wtt = p_ph1.tile([128, IN], F32, tag=f"wt{c}", name=f"wt{c}")
            sync.dma_start(wtt[:], d["wt"].ap()[128 * c:128 * (c + 1), :])
            wt_t.append(wtt)
            w1att = p_ph1.tile([128, H], F32, tag=f"w1at{c}", name=f"w1at{c}")
            sync.dma_start(w1att[:], d["w1at"].ap()[128 * c:128 * (c + 1), :])
            w1at_t.append(w1att)

        psum_a = pp.tile([H, IN], F32)        # 6 banks
        psum_c = pp.tile([H, JL], F32)
        psum_bb = pp.tile([H, JL], F32)
        # row_part partial: contract over local j (k = 384 in 3 chunks)
        for s in range(IN // 512):
            for c in range(3):
                nc.tensor.matmul(psum_a[:, 512 * s:512 * (s + 1)],
                                 w1at_t[c][:], wt_t[c][:, 512 * s:512 * (s + 1)],
                                 start=(c == 0), stop=(c == 2))
        aT_part = p_ph1.tile([H, IN], F32)
        nc.scalar.copy(aT_part[:], psum_a[:])
        # col_part^T: contract over all i
        for t in range(NT):
            nc.tensor.matmul(psum_c[:], w1bt_t[t][:], wc_t[t][:],
                             start=(t == 0), stop=(t == NT - 1))
        # bias broadcast to 10 partitions, tmp = bias*W1c + b1
        nc.tensor.matmul(psum_bb[:], ones10[:], biasj_sb[:])
        nc.vector.tensor_scalar(tmp_sb[:], psum_bb[:], w1c_sb[:], b1c_sb[:],
                                op0=OP.mult, op1=OP.add)
        nc.vector.tensor_tensor(colbT[:], psum_c[:], tmp_sb[:], op=OP.add)

        # AllReduce of row_part partial
        ar_in = p_dram.tile([H, IN], F32)
        ar_out = p_dram.tile([H, IN], F32)
        nc.gpsimd.dma_start(ar_in[:], aT_part[:])
        nc.gpsimd.collective_compute("AllReduce", OP.add, replica_groups=RG,
                                     ins=[ar_in.opt()], outs=[ar_out.opt()])
        nc.gpsimd.dma_start(aT_sb[:], ar_out[:])

    # replicate colbT 12x; pack a
    for m in range(GRP):
        sync.dma_start(cb[10 * m:10 * m + 10, :], colbT[:])
        sync.dma_start(a_pk[10 * m:10 * m + 10, :], aT_sb[:, m::GRP])

    # ---------------- phase 2: pair-grid MLP -> new_w tiles ----------------
    nw_t = []
    for t in range(NT):
        rows = TILE_ROWS[t]
        nwt = p_nw.tile([rows, JL], F32, tag=f"nw{t}", name=f"nw{t}")
        nw_t.append(nwt)

    with tc.tile_pool(name="pph2", space="PSUM", bufs=3) as pp_h2, \
         tc.tile_pool(name="ppw", space="PSUM", bufs=2) as pp_w:
        for t in range(NT):
            rows = TILE_ROWS[t]
            ngrp = rows // GRP
            psw = pp_w.tile([rows, JL], F32, tag="psw", name=f"psw{t}")
            # prefill with weight rows (identity matmul)
            nc.tensor.matmul(psw[:], ident[:rows, :rows], wc_t[t][:],
                             start=True, stop=False)
            for q in range(ngrp):
                g = t * 10 + q
                h1 = p_work.tile([TIL, JL], F32, tag="h1", name=f"h1_{g}", bufs=3)
                nc.vector.tensor_scalar(h1[:], cb[:], a_pk[:, g:g + 1],
                                        0.0, op0=OP.add, op1=OP.max)
                ph2 = pp_h2.tile([TIL, JL], F32, tag="ph2", name=f"ph2_{g}")
                nc.tensor.matmul(ph2[:], bdw2t[:], h1[:],
                                 start=True, stop=True)
                h2 = p_work.tile([TIL, JL], F32, tag="h2", name=f"h2_{g}", bufs=3)
                nc.scalar.activation(h2[:], ph2[:], ACT.Relu, bias=b2t_sb[:])
                off = 108 - GRP * q
                nc.tensor.matmul(psw[:], bw3[:, off:off + rows], h2[:],
                                 start=False, stop=(q == ngrp - 1))
            # new_w = psum (weight + updates) + b3
            nc.scalar.activation(nw_t[t][:], psw[:], ACT.Identity,
                                 bias=b3b_sb[:rows, :])

    # ---------------- phase 3: bias update (local columns) ----------------
    nb_sb = p_const.tile([128, JL], F32)
    with tc.tile_pool(name="ppnb", space="PSUM", bufs=1) as pp_nb:
        ps_g1 = pp_nb.tile([H, JL], F32)
        for t in range(NT):
            nc.tensor.matmul(ps_g1[:], w1bt_t[t][:], nw_t[t][:],
                             start=(t == 0), stop=(t == NT - 1))
        g1z = p_work.tile([H, JL], F32)
        nc.vector.tensor_tensor(g1z[:], ps_g1[:], tmp_sb[:], op=OP.add)
        g1 = p_work.tile([H, JL], F32)
        nc.vector.tensor_scalar(g1[:], g1z[:], 0.0, None, op0=OP.max)
        ps_g2 = pp_nb.tile([H, JL], F32)
        nc.tensor.matmul(ps_g2[:], w2t_sb[:], g1[:], start=True, stop=True)
        g2 = p_work.tile([H, JL], F32)
        nc.scalar.activation(g2[:], ps_g2[:], ACT.Relu, bias=b2c_sb[:])
        ps_db = pp_nb.tile([1, JL], F32)
        nc.tensor.matmul(ps_db[:], w3c_sb[:], g2[:], start=True, stop=True)
        nbrow = p_work.tile([1, JL], F32)
        nc.vector.tensor_tensor(nbrow[:], ps_db[:], biasj_sb[:], op=OP.add)
        nbrow2 = p_work.tile([1, JL], F32)
        nc.vector.tensor_scalar(nbrow2[:], nbrow[:], b3b_sb[:1, :], None, op0=OP.add)
        ps_nb = pp_nb.tile([128, JL], F32)
        nc.tensor.matmul(ps_nb[:], ones128[:], nbrow2[:], start=True, stop=True)
        nc.scalar.copy(nb_sb[:], ps_nb[:])

    # ---------------- phase 4: logits + local softmax stats ----------------
    mx_all = p_const.tile([128, NBT], F32)
    nmx_all = p_const.tile([128, NBT], F32)
    s_all = p_const.tile([128, NBT], F32)
    e_t = [p_e.tile([128, JL], F32, tag=f"e{bt}", name=f"e{bt}") for bt in range(NBT)]

    st_in = p_dram.tile([B, 2], F32)
    st_out = p_dram.tile([NCORES * B, 2], F32)

    with tc.tile_pool(name="ppl", space="PSUM", bufs=2) as pp_l, \
         tc.tile_pool(name="xtp", bufs=2) as p_xt:
        for bt in range(NBT):
            xt_full = p_xt.tile([TIL, 25 * 128], F32, tag="xtf", name=f"xtf{bt}")
            src = d["xtb"].ap()[bt * IN: bt * IN + 25 * TIL, :]
            sync.dma_start(
                xt_full[:].rearrange("p (i c) -> p i c", i=25),
                src.rearrange("(i p) c -> p i c", p=TIL))
            xt_part = p_xt.tile([TILE_ROWS[25], 128], F32, tag="xtp", name=f"xtp{bt}")
            sync.dma_start(xt_part[:], d["xtb"].ap()[bt * IN + 25 * TIL:(bt + 1) * IN, :])

            psl = pp_l.tile([128, JL], F32, tag="psl", name=f"psl{bt}")
            for t in range(NT):
                lhs = xt_full[:, 128 * t:128 * (t + 1)] if t < 25 else xt_part[:]
                nc.tensor.matmul(psl[:], lhs, nw_t[t][:],
                                 start=(t == 0), stop=(t == NT - 1))
            lsb = p_work.tile([128, JL], F32, tag="lsb", name=f"lsb{bt}", bufs=2)
            nc.vector.tensor_tensor(lsb[:], psl[:], nb_sb[:], op=OP.add)
            nc.vector.tensor_reduce(mx_all[:, bt:bt + 1], lsb[:], axis=AX.X, op=OP.max)
            nc.vector.tensor_scalar(nmx_all[:, bt:bt + 1], mx_all[:, bt:bt + 1],
                                    -1.0, None, op0=OP.mult)
            nc.scalar.activation(e_t[bt][:], lsb[:], ACT.Exp,
                                 bias=nmx_all[:, bt:bt + 1],
                                 accum_out=s_all[:, bt:bt + 1])
            nc.gpsimd.dma_start(st_in[128 * bt:128 * (bt + 1), 0:1], mx_all[:, bt:bt + 1])
            nc.gpsimd.dma_start(st_in[128 * bt:128 * (bt + 1), 1:2], s_all[:, bt:bt + 1])

    # ---------------- phase 5: AllGather stats, combine, scale, out ----------------
    nc.gpsimd.collective_compute("AllGather", OP.bypass, replica_groups=RG,
                                 ins=[st_in.opt()], outs=[st_out.opt()])
    ag = p_const.tile([128, NBT * NCORES * 2], F32)
    ag4 = ag[:].rearrange("p (b c k) -> p b c k", b=NBT, c=NCORES)
    for c in range(NCORES):
        sync.dma_start(
            ag4[:, :, c, :],
            st_out[:].rearrange("(c b p) k -> c p b k", c=NCORES, p=128)[c])

    m_gl = p_const.tile([128, NBT], F32)
    s_gl = p_const.tile([128, NBT], F32)
    nc.vector.tensor_reduce(m_gl[:], ag4[:, :, :, 0], axis=AX.X, op=OP.max)
    for c in range(NCORES):
        dif = p_work.tile([128, NBT], F32, tag="dif", name=f"dif{c}", bufs=2)
        nc.vector.tensor_tensor(dif[:], ag4[:, :, c, 0], m_gl[:], op=OP.subtract)
        ex = p_work.tile([128, NBT], F32, tag="ex", name=f"ex{c}", bufs=2)
        nc.scalar.activation(ex[:], dif[:], ACT.Exp)
        wgt = p_work.tile([128, NBT], F32, tag="wgt", name=f"wgt{c}", bufs=2)
        nc.vector.tensor_tensor(wgt[:], ex[:], ag4[:, :, c, 1], op=OP.mult)
        if c == 0:
            nc.vector.tensor_copy(s_gl[:], wgt[:])
        else:
            nc.vector.tensor_tensor(s_gl[:], s_gl[:], wgt[:], op=OP.add)
    rs = p_const.tile([128, NBT], F32)
    nc.vector.reciprocal(rs[:], s_gl[:])
    dif_o = p_const.tile([128, NBT], F32)
    nc.vector.tensor_tensor(dif_o[:], mx_all[:], m_gl[:], op=OP.subtract)
    e_o = p_const.tile([128, NBT], F32)
    nc.scalar.activation(e_o[:], dif_o[:], ACT.Exp)
    scl = p_const.tile([128, NBT], F32)
    nc.vector.tensor_tensor(scl[:], e_o[:], rs[:], op=OP.mult)

    for bt in range(NBT):
        osb = p_work.tile([128, JL], F32, tag="osb", name=f"osb{bt}", bufs=2)
        nc.vector.tensor_scalar(osb[:], e_t[bt][:], scl[:, bt:bt + 1], None, op0=OP.mult)
        sync.dma_start(d["out"].ap()[128 * bt:128 * (bt + 1), :], osb[:])

    if "dbg_neww" in d:
        for t in range(NT):
            sync.dma_start(d["dbg_neww"].ap()[TIL * t:TIL * t + TILE_ROWS[t], :],
                           nw_t[t][:])
        sync.dma_start(d["dbg_newb"].ap()[:], nb_sb[:1, :])
        sync.dma_start(d["dbg_colbt"].ap()[:], colbT[:])

    for p in reversed(ctx_pools):
        p.__exit__(None, None, None)


def build_nc(debug_outs=False):
    nc = bacc.Bacc("TRN2", target_bir_lowering=False, debug=False,
                   num_devices=NCORES)
    d = {}
    for name, shape in [
        ("wcols", [IN, JL]), ("wt", [JL, IN]), ("xtb", [NBT * IN, 128]),
        ("w1at", [JL, H]), ("w1bt", [IN, H]), ("w2t", [H, H]),
        ("w3c", [H, 1]), ("w1c", [H, 1]), ("b1c", [H, 1]), ("b2c", [H, 1]),
        ("b2t", [TIL, 1]), ("b3b", [128, 1]), ("biasj", [1, JL]),
    ]:
        d[name] = nc.dram_tensor(name, shape, F32, kind="ExternalInput")
    d["out"] = nc.dram_tensor("out", [B, JL], F32, kind="ExternalOutput")
    if debug_outs:
        d["dbg_neww"] = nc.dram_tensor("dbg_neww", [IN, JL], F32, kind="ExternalOutput")
        d["dbg_newb"] = nc.dram_tensor("dbg_newb", [1, JL], F32, kind="ExternalOutput")
        d["dbg_colbt"] = nc.dram_tensor("dbg_colbt", [H, JL], F32, kind="ExternalOutput")
    with tile.TileContext(nc) as tc:
        _build_body(nc, tc, d)
    nc.compile()
    return nc


def make_in_maps(X, weight, bias, W1, b1, W2, b2, W3, b3):
    f = lambda a: np.ascontiguousarray(a, dtype=np.float32)
    X, weight, bias = f(X), f(weight), f(bias)
    W1, b1, W2, b2, W3, b3 = f(W1), f(b1), f(W2), f(b2), f(W3), f(b3)
    W1aT = f(W1[:, :OUT].T)          # [3072, 10]
    W1bT = f(W1[:, OUT:OUT + IN].T)  # [3072, 10]
    XT = X.T                          # [3072, 2048]
    xtb = f(XT.reshape(IN, NBT, 128).transpose(1, 0, 2).reshape(NBT * IN, 128))
    wT = f(weight.T)
    w2t = f(W2.T)
    w3c = f(W3.reshape(H, 1))
    w1c = f(W1[:, -1].reshape(H, 1))
    b1c, b2c = f(b1.reshape(H, 1)), f(b2.reshape(H, 1))
    b2t = f(np.tile(b2, GRP).reshape(TIL, 1))
    b3b = np.full((128, 1), b3.reshape(-1)[0], dtype=np.float32)
    in_maps = []
    for c in range(NCORES):
        j0 = c * JL
        in_maps.append({
            "wcols": f(weight[:, j0:j0 + JL]),
            "wt": f(wT[j0:j0 + JL]),
            "xtb": xtb,
            "w1at": f(W1aT[j0:j0 + JL]),
            "w1bt": W1bT,
            "w2t": w2t, "w3c": w3c, "w1c": w1c,
            "b1c": b1c, "b2c": b2c, "b2t": b2t, "b3b": b3b,
            "biasj": f(bias[j0:j0 + JL].reshape(1, JL)),
        })
    return in_maps


_NC_CACHE = {}


def kernel(X, weight, bias, W1, b1, W2, b2, W3, b3):
    global LAST_RESULTS
    if "nc" not in _NC_CACHE:
        _NC_CACHE["nc"] = build_nc()
    nc = _NC_CACHE["nc"]
    in_maps = make_in_maps(X, weight, bias, W1, b1, W2, b2, W3, b3)
    res = bass_utils.run_bass_kernel_spmd(
        nc, in_maps, core_ids=list(range(NCORES)), trace=TRACE)
    LAST_RESULTS = res
    return np.concatenate([res.results[c]["out"] for c in range(NCORES)], axis=1)


# revision 11
# speedup vs baseline: 1.9502x; 1.9502x over previous
"""MetaNCA fused kernel for 8 TRN2 NeuronCores.

Sharding: the [in_units, out_units] pair grid is sharded along out_units (j)
across the 8 cores — each core owns a 384-column block of the weight matrix,
computes its block of the pairwise-MLP updates, the (local-column) bias
update, X @ new_w[:, block] and a j-sharded softmax.  Cross-core traffic is
one AllReduce of the row_part partial ([10, 3072], contracted over local j)
and two AllGathers of per-core softmax stats.

Pair-MLP mapping: groups of 12 i-rows are packed on the partition axis as
(m, h) = 12 rows x 10 hidden units = 120 partitions.  Stage 1 (relu of
row_i + col_j + b1) is a single DVE tensor_scalar per group; stages 2/3 are
PE matmuls with a block-diagonal W2^T and a shifted block-diagonal W3 that
accumulates straight into a PSUM bank pre-filled (by an identity matmul)
with the corresponding weight rows, so new_w = weight + updates materializes
in PSUM for free.

Matmuls run as float32r (single-pass fp32 on the PE; 4x the fp32 rate at
free-dim >= 256) except the weight prefill, which stays exact fp32.
Host-side input re-layout gives every big DMA >= 4KB contiguous
per-partition lines (descriptor-count, not bytes, dominates DMA time).
"""

import numpy as np

import concourse.bass as bass
import concourse.mybir as mybir
import concourse.tile as tile
from concourse import bacc
from concourse import bass_utils
from concourse.masks import make_identity

F32 = mybir.dt.float32
F32R = mybir.dt.float32r
AX = mybir.AxisListType
OP = mybir.AluOpType
ACT = mybir.ActivationFunctionType

IN, OUT, B, H = 3072, 3072, 2048, 10
NCORES = 8
JL = OUT // NCORES          # 384 local out_units per core
GRP = 12                    # i rows packed per group (12*10 = 120 partitions)
TIL = 120                   # i rows per tile (10 groups)
NT = 26                     # ceil(3072/120); last tile has 72 valid rows
NGRP = [10] * 25 + [6]      # groups per tile
NBT = B // 128              # 16 b-tiles
HBT = NBT // 2              # 8 b-tiles per half
RG = [list(range(NCORES))]

TRACE = False               # set by test.py to capture an NTFF trace
LAST_RESULTS = None


def _build_body(nc, tc, d):
    ctx_pools = []

    def pool(name, **kw):
        cm = tc.tile_pool(name=name, **kw)
        p = cm.__enter__()
        ctx_pools.append(cm)
        return p

    p_const = pool("const", bufs=1)
    p_nw = pool("neww", bufs=1)
    p_work = pool("work", bufs=1)
    p_dram = pool("dram", bufs=1, space="DRAM")
    cm_w = tc.tile_pool(name="wcols", bufs=1)
    p_w = cm_w.__enter__()

    sync = nc.sync

    # ---------------- constants / small inputs -> SBUF ----------------
    w2t_sb = p_const.tile([H, H], F32)
    w3c_sb = p_const.tile([H, 1], F32)
    w1c_sb = p_const.tile([H, 1], F32)
    b1c_sb = p_const.tile([H, 1], F32)
    b2c_sb = p_const.tile([H, 1], F32)
    b2t_sb = p_const.tile([TIL, 1], F32)
    b3b_sb = p_const.tile([128, 1], F32)
    biasj_sb = p_const.tile([1, JL], F32R)
    for t, name in [(w2t_sb, "w2t"), (w3c_sb, "w3c"), (w1c_sb, "w1c"),
                    (b1c_sb, "b1c"), (b2c_sb, "b2c"), (b2t_sb, "b2t"),
                    (b3b_sb, "b3b"), (biasj_sb, "biasj")]:
        sync.dma_start(t[:], d[name].ap())

    ones10 = p_const.tile([1, H], F32)
    nc.vector.memset(ones10[:], 1.0)
    ones10_r = p_const.tile([1, H], F32R)
    nc.vector.tensor_scalar(ones10_r[:], ones10[:], 0.0, None, op0=OP.add)
    ones128 = p_const.tile([1, 128], F32)
    nc.vector.memset(ones128[:], 1.0)
    ones128_r = p_const.tile([1, 128], F32R)
    nc.vector.tensor_scalar(ones128_r[:], ones128[:], 0.0, None, op0=OP.add)

    # block-diag W2^T [120,120]: block m rows/cols [10m:10m+10] = W2^T
    bdw2t = p_const.tile([TIL, TIL], F32)
    nc.vector.memset(bdw2t[:], 0.0)
    # shifted block-diag W3 [120, 240]: bw3[10m+g, 108+m] = W3[0, g]
    bw3 = p_const.tile([TIL, 240], F32)
    nc.vector.memset(bw3[:], 0.0)
    for m in range(GRP):
        sync.dma_start(bdw2t[10 * m:10 * m + 10, 10 * m:10 * m + 10], w2t_sb[:])
        sync.dma_start(bw3[10 * m:10 * m + 10, 108 + m:109 + m], w3c_sb[:])
    bdw2t_r = p_const.tile([TIL, TIL], F32R)
    nc.vector.tensor_scalar(bdw2t_r[:], bdw2t[:], 0.0, None, op0=OP.add)
    bw3_r = p_const.tile([TIL, 240], F32R)
    nc.vector.tensor_scalar(bw3_r[:], bw3[:], 0.0, None, op0=OP.add)
    w2t_r = p_const.tile([H, H], F32R)
    nc.vector.tensor_scalar(w2t_r[:], w2t_sb[:], 0.0, None, op0=OP.add)
    w3c_r = p_const.tile([H, 1], F32R)
    nc.vector.tensor_scalar(w3c_r[:], w3c_sb[:], 0.0, None, op0=OP.add)

    # ---------------- big weight loads (re-laid-out, 1 DMA each) -------
    # wcols_all[p, t*JL + j] = weight[120t + p, j0 + j]  (zero-padded t=25)
    wcols_all = p_w.tile([TIL, NT * JL], F32R)
    sync.dma_start(wcols_all[:], d["wcols"].ap())
    wc_t = [wcols_all[:, t * JL:(t + 1) * JL] for t in range(NT)]
    # w1bt_all[p, t*H + h] = W1b[h, 120t + p]  (zero-padded)
    w1bt_all = p_const.tile([TIL, NT * H], F32R)
    sync.dma_start(w1bt_all[:], d["w1bt"].ap())
    w1bt_t = [w1bt_all[:, t * H:(t + 1) * H] for t in range(NT)]

    # ---------------- phase 1: partials + tiny prologue math ----------------
    aT_sb = p_const.tile([H, IN], F32)        # row_part^T after AllReduce
    colbT = p_const.tile([H, JL], F32)        # col_part^T + bias*W1c + b1
    tmp_sb = p_const.tile([H, JL], F32)       # bias*W1c + b1  (reused for new_b)
    cb = p_const.tile([TIL, JL], F32)         # colbT replicated 12x
    a_pk = p_const.tile([TIL, IN // GRP], F32)  # a packed [120, 256]

    with tc.tile_pool(name="ppearly", space="PSUM", bufs=1) as pp, \
         tc.tile_pool(name="ph1", bufs=1) as p_ph1:
        wt_t = []
        w1at_t = []
        for c in range(3):
            wtt = p_ph1.tile([128, IN], F32R, tag=f"wt{c}", name=f"wt{c}")
            sync.dma_start(wtt[:], d["wt"].ap()[128 * c:128 * (c + 1), :])
            wt_t.append(wtt)
            w1att = p_ph1.tile([128, H], F32R, tag=f"w1at{c}", name=f"w1at{c}")
            sync.dma_start(w1att[:], d["w1at"].ap()[128 * c:128 * (c + 1), :])
            w1at_t.append(w1att)

        psum_a = pp.tile([H, IN], F32)        # 6 banks
        psum_c = pp.tile([H, JL], F32)
        psum_bb = pp.tile([H, JL], F32)
        # row_part partial: contract over local j (k = 384 in 3 chunks)
        for s in range(IN // 512):
            for c in range(3):
                nc.tensor.matmul(psum_a[:, 512 * s:512 * (s + 1)],
                                 w1at_t[c][:],
                                 wt_t[c][:, 512 * s:512 * (s + 1)],
                                 start=(c == 0), stop=(c == 2))
        aT_part = p_ph1.tile([H, IN], F32)
        nc.scalar.copy(aT_part[:], psum_a[:])

        # AllReduce of row_part partial (fires early; col_part overlaps it)
        ar_in = p_dram.tile([H, IN], F32)
        ar_out = p_dram.tile([H, IN], F32)
        nc.gpsimd.dma_start(ar_in[:], aT_part[:])
        nc.gpsimd.collective_compute("AllReduce", OP.add, replica_groups=RG,
                                     ins=[ar_in.opt()], outs=[ar_out.opt()])
        nc.gpsimd.dma_start(aT_sb[:], ar_out[:])

        # col_part^T: contract over all i (local work, overlaps the AR)
        for t in range(NT):
            nc.tensor.matmul(psum_c[:], w1bt_t[t], wc_t[t],
                             start=(t == 0), stop=(t == NT - 1))
        # bias broadcast to 10 partitions, tmp = bias*W1c + b1
        nc.tensor.matmul(psum_bb[:], ones10_r[:], biasj_sb[:])
        nc.vector.tensor_scalar(tmp_sb[:], psum_bb[:], w1c_sb[:], b1c_sb[:],
                                op0=OP.mult, op1=OP.add)
        nc.vector.tensor_tensor(colbT[:], psum_c[:], tmp_sb[:], op=OP.add)

    # replicate colbT 12x; pack a
    for m in range(GRP):
        sync.dma_start(cb[10 * m:10 * m + 10, :], colbT[:])
        sync.dma_start(a_pk[10 * m:10 * m + 10, :], aT_sb[:, m::GRP])

    # ---------------- phase 2: pair-grid MLP -> new_w tiles ----------------
    nw_t = [p_nw.tile([TIL, JL], F32R, tag=f"nw{t}", name=f"nw{t}")
            for t in range(NT)]

    with tc.tile_pool(name="pph2", space="PSUM", bufs=3) as pp_h2, \
         tc.tile_pool(name="ppw", space="PSUM", bufs=2) as pp_w:
        for t in range(NT):
            ngrp = NGRP[t]
            psw = pp_w.tile([TIL, JL], F32, tag="psw", name=f"psw{t}")
            for q in range(ngrp):
                g = t * 10 + q
                h1 = p_work.tile([TIL, JL], F32R, tag="h1", name=f"h1_{g}", bufs=3)
                nc.vector.tensor_scalar(h1[:], cb[:], a_pk[:, g:g + 1],
                                        0.0, op0=OP.add, op1=OP.max)
                ph2 = pp_h2.tile([TIL, JL], F32, tag="ph2", name=f"ph2_{g}")
                nc.tensor.matmul(ph2[:], bdw2t_r[:], h1[:],
                                 start=True, stop=True)
                h2 = p_work.tile([TIL, JL], F32R, tag="h2", name=f"h2_{g}", bufs=3)
                nc.scalar.activation(h2[:], ph2[:], ACT.Relu, bias=b2t_sb[:])
                off = 108 - GRP * q
                nc.tensor.matmul(psw[:], bw3_r[:, off:off + TIL], h2[:],
                                 start=(q == 0), stop=(q == ngrp - 1))
            # new_w = weight + updates + b3 (exact fp32 add, rounded to f32r)
            upd = p_work.tile([TIL, JL], F32, tag="upd", name=f"upd{t}", bufs=2)
            nc.scalar.activation(upd[:], psw[:], ACT.Identity,
                                 bias=b3b_sb[:TIL, :])
            nc.vector.tensor_tensor(nw_t[t][:], upd[:], wc_t[t].bitcast(F32),
                                    op=OP.add)

    cm_w.__exit__(None, None, None)

    # ---------------- phase 3: bias update (local columns) ----------------
    nb_sb = p_const.tile([128, JL], F32)
    with tc.tile_pool(name="ppnb", space="PSUM", bufs=1) as pp_nb:
        ps_g1 = pp_nb.tile([H, JL], F32)
        for t in range(NT):
            nc.tensor.matmul(ps_g1[:], w1bt_t[t], nw_t[t][:],
                             start=(t == 0), stop=(t == NT - 1))
        g1z = p_work.tile([H, JL], F32)
        nc.vector.tensor_tensor(g1z[:], ps_g1[:], tmp_sb[:], op=OP.add)
        g1 = p_work.tile([H, JL], F32R)
        nc.vector.tensor_scalar(g1[:], g1z[:], 0.0, None, op0=OP.max)
        ps_g2 = pp_nb.tile([H, JL], F32)
        nc.tensor.matmul(ps_g2[:], w2t_r[:], g1[:], start=True, stop=True)
        g2 = p_work.tile([H, JL], F32R)
        nc.scalar.activation(g2[:], ps_g2[:], ACT.Relu, bias=b2c_sb[:])
        ps_db = pp_nb.tile([1, JL], F32)
        nc.tensor.matmul(ps_db[:], w3c_r[:], g2[:], start=True, stop=True)
        nbrow = p_work.tile([1, JL], F32)
        nc.vector.tensor_tensor(nbrow[:], ps_db[:], biasj_sb[:].bitcast(F32), op=OP.add)
        nbrow2 = p_work.tile([1, JL], F32R)
        nc.vector.tensor_scalar(nbrow2[:], nbrow[:], b3b_sb[:1, :], None, op0=OP.add)
        ps_nb = pp_nb.tile([128, JL], F32)
        nc.tensor.matmul(ps_nb[:], ones128_r[:], nbrow2[:], start=True, stop=True)
        nc.scalar.copy(nb_sb[:], ps_nb[:])

    # ------- phase 4: logits + softmax, two b-halves of 8 b-tiles each -----
    p_e = pool("etiles", bufs=1)
    # per-half stats layout [128, 16]: cols [0:8] = mx, [8:16] = s
    for half in range(2):
        stats = p_work.tile([128, 2 * HBT], F32, tag="st", name=f"st{half}", bufs=2)
        nmx = p_work.tile([128, HBT], F32, tag=f"nmx{half}", name=f"nmx{half}")
        e_t = [p_e.tile([128, JL], F32, tag=f"e_{b}", name=f"e{half}_{b}")
               for b in range(HBT)]
        st_in = p_dram.tile([128, 2 * HBT], F32, name=f"stin{half}")
        st_out = p_dram.tile([NCORES * 128, 2 * HBT], F32, name=f"stout{half}")

        with tc.tile_pool(name=f"ppl{half}", space="PSUM", bufs=1) as pp_l, \
             tc.tile_pool(name=f"xtp{half}", bufs=3) as p_xt:
            psl = [pp_l.tile([128, JL], F32, tag=f"psl{b}", name=f"psl{half}_{b}")
                   for b in range(HBT)]
            for t in range(NT):
                xt = p_xt.tile([TIL, 1024], F32R, tag="xt", name=f"xt{half}_{t}")
                off = t * B + half * 1024
                sync.dma_start(xt[:], d["xtr"].ap()[:, off:off + 1024])
                for b in range(HBT):
                    nc.tensor.matmul(psl[b][:], xt[:, 128 * b:128 * (b + 1)],
                                     nw_t[t][:],
                                     start=(t == 0), stop=(t == NT - 1))
            for b in range(HBT):
                lsb = p_work.tile([128, JL], F32, tag="lsb", name=f"lsb{half}_{b}",
                                  bufs=2)
                nc.vector.tensor_tensor(lsb[:], psl[b][:], nb_sb[:], op=OP.add)
                nc.vector.tensor_reduce(stats[:, b:b + 1], lsb[:], axis=AX.X,
                                        op=OP.max)
                nc.vector.tensor_scalar(nmx[:, b:b + 1], stats[:, b:b + 1],
                                        -1.0, None, op0=OP.mult)
                nc.scalar.activation(e_t[b][:], lsb[:], ACT.Exp,
                                     bias=nmx[:, b:b + 1],
                                     accum_out=stats[:, HBT + b:HBT + b + 1])
        sync.dma_start(st_in[:], stats[:])
        nc.gpsimd.collective_compute("AllGather", OP.bypass, replica_groups=RG,
                                     ins=[st_in.opt()], outs=[st_out.opt()])
        # readback [128, (c, 16)]
        ag = p_work.tile([128, NCORES * 2 * HBT], F32, tag="ag",
                          name=f"ag{half}", bufs=2)
        sync.dma_start(
            ag[:].rearrange("p (c k) -> p c k", c=NCORES),
            st_out[:].rearrange("(c p) k -> p c k", c=NCORES))
        ag3 = ag[:].rearrange("p (c k) -> p c k", c=NCORES)

        m_gl = p_work.tile([128, HBT], F32, tag="mgl", name=f"mgl{half}", bufs=2)
        # max over cores: view [128, bt, c] with bt stride 1, c stride 16
        nc.vector.tensor_reduce(
            m_gl[:], ag3[:, :, 0:HBT].rearrange("p c b -> p b c"),
            axis=AX.X, op=OP.max)
        s_gl = p_work.tile([128, HBT], F32, tag="sgl", name=f"sgl{half}", bufs=2)
        for c in range(NCORES):
            dif = p_work.tile([128, HBT], F32, tag="dif", name=f"dif{half}_{c}",
                              bufs=2)
            nc.vector.tensor_tensor(dif[:], ag3[:, c, 0:HBT], m_gl[:],
                                    op=OP.subtract)
            ex = p_work.tile([128, HBT], F32, tag="ex", name=f"ex{half}_{c}",
                             bufs=2)
            nc.scalar.activation(ex[:], dif[:], ACT.Exp)
            wgt = p_work.tile([128, HBT], F32, tag="wgt", name=f"wgt{half}_{c}",
                              bufs=2)
            nc.vector.tensor_tensor(wgt[:], ex[:], ag3[:, c, HBT:2 * HBT],
                                    op=OP.mult)
            if c == 0:
                nc.vector.tensor_copy(s_gl[:], wgt[:])
            else:
                nc.vector.tensor_tensor(s_gl[:], s_gl[:], wgt[:], op=OP.add)
        rs = p_work.tile([128, HBT], F32, tag="rs", name=f"rs{half}", bufs=2)
        nc.vector.reciprocal(rs[:], s_gl[:])
        dif_o = p_work.tile([128, HBT], F32, tag="difo", name=f"difo{half}", bufs=2)
        nc.vector.tensor_tensor(dif_o[:], stats[:, 0:HBT], m_gl[:], op=OP.subtract)
        e_o = p_work.tile([128, HBT], F32, tag="eo", name=f"eo{half}", bufs=2)
        nc.scalar.activation(e_o[:], dif_o[:], ACT.Exp)
        scl = p_work.tile([128, HBT], F32, tag="scl", name=f"scl{half}", bufs=2)
        nc.vector.tensor_tensor(scl[:], e_o[:], rs[:], op=OP.mult)

        oslab = p_e.tile([128, HBT * JL], F32, tag="os", name=f"os{half}")
        for b in range(HBT):
            nc.vector.tensor_scalar(oslab[:, b * JL:(b + 1) * JL], e_t[b][:],
                                    scl[:, b:b + 1], None, op0=OP.mult)
        sync.dma_start(d["out"].ap()[:, half * HBT * JL:(half + 1) * HBT * JL],
                       oslab[:])

    for p in reversed(ctx_pools):
        p.__exit__(None, None, None)


def build_nc():
    nc = bacc.Bacc("TRN2", target_bir_lowering=False, debug=False,
                   num_devices=NCORES)
    d = {}
    for name, shape, dt_ in [
        ("wcols", [TIL, NT * JL], F32R), ("wt", [JL, IN], F32R),
        ("xtr", [TIL, NT * B], F32R),
        ("w1at", [JL, H], F32R), ("w1bt", [TIL, NT * H], F32R),
        ("w2t", [H, H], F32),
        ("w3c", [H, 1], F32), ("w1c", [H, 1], F32), ("b1c", [H, 1], F32),
        ("b2c", [H, 1], F32),
        ("b2t", [TIL, 1], F32), ("b3b", [128, 1], F32), ("biasj", [1, JL], F32R),
    ]:
        d[name] = nc.dram_tensor(name, shape, dt_, kind="ExternalInput")
    d["out"] = nc.dram_tensor("out", [128, NBT * JL], F32, kind="ExternalOutput")
    with tile.TileContext(nc) as tc:
        _build_body(nc, tc, d)
    nc.compile()
    return nc


def _tiled_pad(a, axis_len=IN):
    """[3072, k] -> [120, 26*k] with a[120t+p, :] at [p, t*k:(t+1)*k], zero pad."""
    k = a.shape[1]
    pad = np.zeros((NT * TIL, k), dtype=np.float32)
    pad[:axis_len] = a
    return np.ascontiguousarray(
        pad.reshape(NT, TIL, k).transpose(1, 0, 2).reshape(TIL, NT * k))


def make_in_maps(X, weight, bias, W1, b1, W2, b2, W3, b3):
    f = lambda a: np.ascontiguousarray(a, dtype=np.float32)
    X, weight, bias = f(X), f(weight), f(bias)
    W1, b1, W2, b2, W3, b3 = f(W1), f(b1), f(W2), f(b2), f(W3), f(b3)
    W1aT = f(W1[:, :OUT].T)          # [3072, 10]
    W1bT = f(W1[:, OUT:OUT + IN].T)  # [3072, 10]
    xtr = _tiled_pad(f(X.T))          # [120, 26*2048]
    wT = f(weight.T)
    w1bt = _tiled_pad(W1bT)           # [120, 260]
    w2t = f(W2.T)
    w3c = f(W3.reshape(H, 1))
    w1c = f(W1[:, -1].reshape(H, 1))
    b1c, b2c = f(b1.reshape(H, 1)), f(b2.reshape(H, 1))
    b2t = f(np.tile(b2, GRP).reshape(TIL, 1))
    b3b = np.full((128, 1), b3.reshape(-1)[0], dtype=np.float32)
    in_maps = []
    for c in range(NCORES):
        j0 = c * JL
        in_maps.append({
            "wcols": _tiled_pad(f(weight[:, j0:j0 + JL])),
            "wt": f(wT[j0:j0 + JL]),
            "xtr": xtr,
            "w1at": f(W1aT[j0:j0 + JL]),
            "w1bt": w1bt,
            "w2t": w2t, "w3c": w3c, "w1c": w1c,
            "b1c": b1c, "b2c": b2c, "b2t": b2t, "b3b": b3b,
            "biasj": f(bias[j0:j0 + JL].reshape(1, JL)),
        })
    return in_maps


_NC_CACHE = {}


def kernel(X, weight, bias, W1, b1, W2, b2, W3, b3):
    global LAST_RESULTS
    if "nc" not in _NC_CACHE:
        _NC_CACHE["nc"] = build_nc()
    nc = _NC_CACHE["nc"]
    in_maps = make_in_maps(X, weight, bias, W1, b1, W2, b2, W3, b3)
    res = bass_utils.run_bass_kernel_spmd(
        nc, in_maps, core_ids=list(range(NCORES)), trace=TRACE)
    LAST_RESULTS = res
    blocks = []
    for c in range(NCORES):
        o = res.results[c]["out"]                      # [128, 16*384]
        blocks.append(o.reshape(128, NBT, JL).transpose(1, 0, 2).reshape(B, JL))
    return np.concatenate(blocks, axis=1)


# revision 12
# speedup vs baseline: 2.2467x; 1.1520x over previous
"""MetaNCA fused kernel for 8 TRN2 NeuronCores.

Sharding: the [in_units, out_units] pair grid is sharded along out_units (j)
across the 8 cores — each core owns a 384-column block of the weight matrix,
computes its block of the pairwise-MLP updates, the (local-column) bias
update, X @ new_w[:, block] and a j-sharded softmax.  Cross-core traffic is
one AllReduce of the row_part partial ([10, 3072], contracted over local j)
and two AllGathers of per-core softmax stats.

Pair-MLP mapping: groups of 12 i-rows are packed on the partition axis as
(m, h) = 12 rows x 10 hidden units = 120 partitions.  Stage 1 (relu of
row_i + col_j + b1) is a single DVE tensor_scalar per group; stages 2/3 are
PE matmuls with a block-diagonal W2^T and a shifted block-diagonal W3 that
accumulates straight into a PSUM bank pre-filled (by an identity matmul)
with the corresponding weight rows, so new_w = weight + updates materializes
in PSUM for free.

Matmuls run as float32r (single-pass fp32 on the PE; 4x the fp32 rate at
free-dim >= 256) except the weight prefill, which stays exact fp32.
Host-side input re-layout gives every big DMA >= 4KB contiguous
per-partition lines (descriptor-count, not bytes, dominates DMA time).
"""

import numpy as np

import concourse.bass as bass
import concourse.mybir as mybir
import concourse.tile as tile
from concourse import bacc
from concourse import bass_utils

F32 = mybir.dt.float32
BF16 = mybir.dt.bfloat16
F32R = mybir.dt.float32r
AX = mybir.AxisListType
OP = mybir.AluOpType
ACT = mybir.ActivationFunctionType

IN, OUT, B, H = 3072, 3072, 2048, 10
NCORES = 8
JL = OUT // NCORES          # 384 local out_units per core
GRP = 12                    # i rows packed per group (12*10 = 120 partitions)
TIL = 120                   # i rows per tile (10 groups)
NT = 26                     # ceil(3072/120); last tile has 72 valid rows
NGRP = [10] * 25 + [6]      # groups per tile
NBT = B // 128              # 16 b-tiles
HBT = NBT // 2              # 8 b-tiles per half
RG = [list(range(NCORES))]

TRACE = False               # set by test.py to capture an NTFF trace
LAST_RESULTS = None


def _build_body(nc, tc, d):
    ctx_pools = []

    def pool(name, **kw):
        cm = tc.tile_pool(name=name, **kw)
        p = cm.__enter__()
        ctx_pools.append(cm)
        return p

    p_const = pool("const", bufs=1)
    p_nw = pool("neww", bufs=1)
    p_work = pool("work", bufs=1)
    p_dram = pool("dram", bufs=1, space="DRAM")
    cm_w = tc.tile_pool(name="wcols", bufs=1)
    p_w = cm_w.__enter__()

    sync = nc.sync

    # warm up ncfw / absorb the first-collective entry barrier at t~0
    warm_in = p_dram.tile([1, 8], F32)
    warm_out = p_dram.tile([1, 8], F32)
    warm_sb = p_const.tile([1, 8], F32)
    nc.vector.memset(warm_sb[:], 1.0)
    nc.gpsimd.dma_start(warm_in[:], warm_sb[:])
    nc.gpsimd.collective_compute("AllReduce", OP.add, replica_groups=RG,
                                 ins=[warm_in.opt()], outs=[warm_out.opt()])

    # ---------------- constants / small inputs -> SBUF ----------------
    w2t_sb = p_const.tile([H, H], F32)
    w3c_sb = p_const.tile([H, 1], F32)
    w1c_sb = p_const.tile([H, 1], F32)
    b1c_sb = p_const.tile([H, 1], F32)
    b2c_sb = p_const.tile([H, 1], F32)
    b2t_sb = p_const.tile([TIL, 1], F32)
    b3b_sb = p_const.tile([128, 1], F32)
    biasj_sb = p_const.tile([1, JL], F32R)
    for t, name in [(w2t_sb, "w2t"), (w3c_sb, "w3c"), (w1c_sb, "w1c"),
                    (b1c_sb, "b1c"), (b2c_sb, "b2c"), (b2t_sb, "b2t"),
                    (b3b_sb, "b3b"), (biasj_sb, "biasj")]:
        sync.dma_start(t[:], d[name].ap())

    ones10 = p_const.tile([1, H], F32)
    nc.vector.memset(ones10[:], 1.0)
    ones10_r = p_const.tile([1, H], F32R)
    nc.vector.tensor_scalar(ones10_r[:], ones10[:], 0.0, None, op0=OP.add)
    ones128 = p_const.tile([1, 128], F32)
    nc.vector.memset(ones128[:], 1.0)
    ones128_r = p_const.tile([1, 128], F32R)
    nc.vector.tensor_scalar(ones128_r[:], ones128[:], 0.0, None, op0=OP.add)

    # block-diag W2^T [120,120]: block m rows/cols [10m:10m+10] = W2^T
    bdw2t = p_const.tile([TIL, TIL], F32)
    nc.vector.memset(bdw2t[:], 0.0)
    # shifted block-diag W3 [120, 240]: bw3[10m+g, 108+m] = W3[0, g]
    bw3 = p_const.tile([TIL, 240], F32)
    nc.vector.memset(bw3[:], 0.0)
    for m in range(GRP):
        sync.dma_start(bdw2t[10 * m:10 * m + 10, 10 * m:10 * m + 10], w2t_sb[:])
        sync.dma_start(bw3[10 * m:10 * m + 10, 108 + m:109 + m], w3c_sb[:])
    bdw2t_r = p_const.tile([TIL, TIL], F32R)
    nc.vector.tensor_scalar(bdw2t_r[:], bdw2t[:], 0.0, None, op0=OP.add)
    bw3_r = p_const.tile([TIL, 240], F32R)
    nc.vector.tensor_scalar(bw3_r[:], bw3[:], 0.0, None, op0=OP.add)
    w2t_r = p_const.tile([H, H], F32R)
    nc.vector.tensor_scalar(w2t_r[:], w2t_sb[:], 0.0, None, op0=OP.add)
    w3c_r = p_const.tile([H, 1], F32R)
    nc.vector.tensor_scalar(w3c_r[:], w3c_sb[:], 0.0, None, op0=OP.add)

    # ---------------- big weight loads (re-laid-out, 1 DMA each) -------
    # wcols_all[p, t*JL + j] = weight[120t + p, j0 + j]  (zero-padded t=25)
    wcols_all = p_w.tile([TIL, NT * JL], F32R)
    sync.dma_start(wcols_all[:], d["wcols"].ap())
    wc_t = [wcols_all[:, t * JL:(t + 1) * JL] for t in range(NT)]
    # w1bt_all[p, t*H + h] = W1b[h, 120t + p]  (zero-padded)
    w1bt_all = p_const.tile([TIL, NT * H], F32R)
    sync.dma_start(w1bt_all[:], d["w1bt"].ap())
    w1bt_t = [w1bt_all[:, t * H:(t + 1) * H] for t in range(NT)]

    # ---------------- phase 1: partials + tiny prologue math ----------------
    aT_sb = p_const.tile([H, IN], F32)        # row_part^T after AllReduce
    colbT = p_const.tile([H, JL], F32)        # col_part^T + bias*W1c + b1
    tmp_sb = p_const.tile([H, JL], F32)       # bias*W1c + b1  (reused for new_b)
    cb = p_const.tile([TIL, JL], F32)         # colbT replicated 12x
    a_pk = p_const.tile([TIL, IN // GRP], F32)  # a packed [120, 256]

    with tc.tile_pool(name="ppearly", space="PSUM", bufs=1) as pp, \
         tc.tile_pool(name="ph1", bufs=1) as p_ph1:
        wt_t = []
        w1at_t = []
        for c in range(3):
            wtt = p_ph1.tile([128, IN], F32R, tag=f"wt{c}", name=f"wt{c}")
            sync.dma_start(wtt[:], d["wt"].ap()[128 * c:128 * (c + 1), :])
            wt_t.append(wtt)
            w1att = p_ph1.tile([128, H], F32R, tag=f"w1at{c}", name=f"w1at{c}")
            sync.dma_start(w1att[:], d["w1at"].ap()[128 * c:128 * (c + 1), :])
            w1at_t.append(w1att)

        psum_a = pp.tile([H, IN], F32)        # 6 banks
        psum_c = pp.tile([H, JL], F32)
        psum_bb = pp.tile([H, JL], F32)
        # row_part partial: contract over local j (k = 384 in 3 chunks)
        for s in range(IN // 512):
            for c in range(3):
                nc.tensor.matmul(psum_a[:, 512 * s:512 * (s + 1)],
                                 w1at_t[c][:],
                                 wt_t[c][:, 512 * s:512 * (s + 1)],
                                 start=(c == 0), stop=(c == 2))
        aT_part = p_ph1.tile([H, IN], BF16)
        nc.scalar.activation(aT_part[:], psum_a[:], ACT.Copy)

        # AllReduce (bf16) of row_part partial; col_part overlaps it
        ar_in = p_dram.tile([H, IN], BF16)
        ar_out = p_dram.tile([H, IN], BF16)
        nc.gpsimd.dma_start(ar_in[:], aT_part[:])
        nc.gpsimd.collective_compute("AllReduce", OP.add, replica_groups=RG,
                                     ins=[ar_in.opt()], outs=[ar_out.opt()])
        aT_bf = p_ph1.tile([H, IN], BF16)
        nc.gpsimd.dma_start(aT_bf[:], ar_out[:])
        nc.vector.tensor_copy(aT_sb[:], aT_bf[:])

        # col_part^T: contract over all i (local work, overlaps the AR)
        for t in range(NT):
            nc.tensor.matmul(psum_c[:], w1bt_t[t], wc_t[t],
                             start=(t == 0), stop=(t == NT - 1))
        # bias broadcast to 10 partitions, tmp = bias*W1c + b1
        nc.tensor.matmul(psum_bb[:], ones10_r[:], biasj_sb[:])
        nc.vector.tensor_scalar(tmp_sb[:], psum_bb[:], w1c_sb[:], b1c_sb[:],
                                op0=OP.mult, op1=OP.add)
        nc.vector.tensor_tensor(colbT[:], psum_c[:], tmp_sb[:], op=OP.add)

    # replicate colbT 12x; pack a
    for m in range(GRP):
        sync.dma_start(cb[10 * m:10 * m + 10, :], colbT[:])
        sync.dma_start(a_pk[10 * m:10 * m + 10, :], aT_sb[:, 256 * m:256 * (m + 1)])

    # ---------------- phase 2: pair-grid MLP -> new_w tiles ----------------
    nw_t = [p_nw.tile([TIL, JL], F32R, tag=f"nw{t}", name=f"nw{t}")
            for t in range(NT)]

    with tc.tile_pool(name="pph2", space="PSUM", bufs=3) as pp_h2, \
         tc.tile_pool(name="ppw", space="PSUM", bufs=2) as pp_w:
        for t in range(NT):
            ngrp = NGRP[t]
            psw = pp_w.tile([TIL, JL], F32, tag="psw", name=f"psw{t}")
            for q in range(ngrp):
                g = t * 10 + q
                h1 = p_work.tile([TIL, JL], F32R, tag="h1", name=f"h1_{g}", bufs=3)
                nc.vector.tensor_scalar(h1[:], cb[:], a_pk[:, g:g + 1],
                                        0.0, op0=OP.add, op1=OP.max)
                ph2 = pp_h2.tile([TIL, JL], F32, tag="ph2", name=f"ph2_{g}")
                nc.tensor.matmul(ph2[:], bdw2t_r[:], h1[:],
                                 start=True, stop=True)
                h2 = p_work.tile([TIL, JL], F32R, tag="h2", name=f"h2_{g}", bufs=3)
                nc.scalar.activation(h2[:], ph2[:], ACT.Relu, bias=b2t_sb[:])
                off = 108 - GRP * q
                nc.tensor.matmul(psw[:], bw3_r[:, off:off + TIL], h2[:],
                                 start=(q == 0), stop=(q == ngrp - 1))
            # new_w = weight + updates + b3 (exact fp32 add, rounded to f32r)
            upd = p_work.tile([TIL, JL], F32, tag="upd", name=f"upd{t}", bufs=2)
            nc.scalar.activation(upd[:], psw[:], ACT.Identity,
                                 bias=b3b_sb[:TIL, :])
            nc.vector.tensor_tensor(nw_t[t][:], upd[:], wc_t[t].bitcast(F32),
                                    op=OP.add)

    cm_w.__exit__(None, None, None)

    # ---------------- phase 3: bias update (local columns) ----------------
    nb_sb = p_const.tile([128, JL], F32)
    with tc.tile_pool(name="ppnb", space="PSUM", bufs=1) as pp_nb:
        ps_g1 = pp_nb.tile([H, JL], F32)
        for t in range(NT):
            nc.tensor.matmul(ps_g1[:], w1bt_t[t], nw_t[t][:],
                             start=(t == 0), stop=(t == NT - 1))
        g1z = p_work.tile([H, JL], F32)
        nc.vector.tensor_tensor(g1z[:], ps_g1[:], tmp_sb[:], op=OP.add)
        g1 = p_work.tile([H, JL], F32R)
        nc.vector.tensor_scalar(g1[:], g1z[:], 0.0, None, op0=OP.max)
        ps_g2 = pp_nb.tile([H, JL], F32)
        nc.tensor.matmul(ps_g2[:], w2t_r[:], g1[:], start=True, stop=True)
        g2 = p_work.tile([H, JL], F32R)
        nc.scalar.activation(g2[:], ps_g2[:], ACT.Relu, bias=b2c_sb[:])
        ps_db = pp_nb.tile([1, JL], F32)
        nc.tensor.matmul(ps_db[:], w3c_r[:], g2[:], start=True, stop=True)
        nbrow = p_work.tile([1, JL], F32)
        nc.vector.tensor_tensor(nbrow[:], ps_db[:], biasj_sb[:].bitcast(F32), op=OP.add)
        nbrow2 = p_work.tile([1, JL], F32R)
        nc.vector.tensor_scalar(nbrow2[:], nbrow[:], b3b_sb[:1, :], None, op0=OP.add)
        ps_nb = pp_nb.tile([128, JL], F32)
        nc.tensor.matmul(ps_nb[:], ones128_r[:], nbrow2[:], start=True, stop=True)
        nc.scalar.copy(nb_sb[:], ps_nb[:])

    # ------- phase 4: logits + softmax, two b-halves of 8 b-tiles each -----
    p_e = pool("etiles", bufs=1)
    # per-half stats layout [128, 16]: cols [0:8] = mx, [8:16] = s
    for half in range(2):
        stats = p_work.tile([128, 2 * HBT], F32, tag="st", name=f"st{half}", bufs=2)
        nmx = p_work.tile([128, HBT], F32, tag=f"nmx{half}", name=f"nmx{half}")
        e_t = [p_e.tile([128, JL], F32, tag=f"e_{b}", name=f"e{half}_{b}")
               for b in range(HBT)]
        mx_in = p_dram.tile([128, HBT], F32, name=f"mxin{half}")
        mx_out = p_dram.tile([128, HBT], F32, name=f"mxout{half}")
        sp_in = p_dram.tile([128, HBT], F32, name=f"spin{half}")
        sp_out = p_dram.tile([128, HBT], F32, name=f"spout{half}")

        with tc.tile_pool(name=f"ppl{half}", space="PSUM", bufs=1) as pp_l, \
             tc.tile_pool(name=f"xtp{half}", bufs=6) as p_xt:
            psl = [pp_l.tile([128, JL], F32, tag=f"psl{b}", name=f"psl{half}_{b}")
                   for b in range(HBT)]
            for t in range(NT):
                xt = p_xt.tile([TIL, 1024], F32R, tag="xt", name=f"xt{half}_{t}")
                off = t * B + half * 1024
                sync.dma_start(xt[:], d["xtr"].ap()[:, off:off + 1024])
                for b in range(HBT):
                    nc.tensor.matmul(psl[b][:], xt[:, 128 * b:128 * (b + 1)],
                                     nw_t[t][:],
                                     start=(t == 0), stop=(t == NT - 1))
            for b in range(HBT):
                lsb = p_work.tile([128, JL], F32, tag="lsb", name=f"lsb{half}_{b}",
                                  bufs=2)
                nc.vector.tensor_tensor(lsb[:], psl[b][:], nb_sb[:], op=OP.add)
                nc.vector.tensor_reduce(stats[:, b:b + 1], lsb[:], axis=AX.X,
                                        op=OP.max)
                nc.vector.tensor_scalar(nmx[:, b:b + 1], stats[:, b:b + 1],
                                        -1.0, None, op0=OP.mult)
                nc.scalar.activation(e_t[b][:], lsb[:], ACT.Exp,
                                     bias=nmx[:, b:b + 1],
                                     accum_out=stats[:, HBT + b:HBT + b + 1])
        nc.gpsimd.dma_start(mx_in[:], stats[:, 0:HBT])
        nc.gpsimd.collective_compute("AllReduce", OP.max, replica_groups=RG,
                                     ins=[mx_in.opt()], outs=[mx_out.opt()])
        m_gl = p_work.tile([128, HBT], F32, tag="mgl", name=f"mgl{half}", bufs=2)
        nc.gpsimd.dma_start(m_gl[:], mx_out[:])
        dif_o = p_work.tile([128, HBT], F32, tag="difo", name=f"difo{half}", bufs=2)
        nc.vector.tensor_tensor(dif_o[:], stats[:, 0:HBT], m_gl[:], op=OP.subtract)
        e_o = p_work.tile([128, HBT], F32, tag="eo", name=f"eo{half}", bufs=2)
        nc.scalar.activation(e_o[:], dif_o[:], ACT.Exp)
        sp = p_work.tile([128, HBT], F32, tag="sp", name=f"sp{half}", bufs=2)
        nc.vector.tensor_tensor(sp[:], e_o[:], stats[:, HBT:2 * HBT], op=OP.mult)
        nc.gpsimd.dma_start(sp_in[:], sp[:])
        nc.gpsimd.collective_compute("AllReduce", OP.add, replica_groups=RG,
                                     ins=[sp_in.opt()], outs=[sp_out.opt()])
        s_gl = p_work.tile([128, HBT], F32, tag="sgl", name=f"sgl{half}", bufs=2)
        nc.gpsimd.dma_start(s_gl[:], sp_out[:])
        rs = p_work.tile([128, HBT], F32, tag="rs", name=f"rs{half}", bufs=2)
        nc.vector.reciprocal(rs[:], s_gl[:])
        scl = p_work.tile([128, HBT], F32, tag="scl", name=f"scl{half}", bufs=2)
        nc.vector.tensor_tensor(scl[:], e_o[:], rs[:], op=OP.mult)

        oslab = p_e.tile([128, HBT * JL], F32, tag="os", name=f"os{half}")
        for b in range(HBT):
            nc.vector.tensor_scalar(oslab[:, b * JL:(b + 1) * JL], e_t[b][:],
                                    scl[:, b:b + 1], None, op0=OP.mult)
        sync.dma_start(d["out"].ap()[:, half * HBT * JL:(half + 1) * HBT * JL],
                       oslab[:])

    for p in reversed(ctx_pools):
        p.__exit__(None, None, None)


def build_nc():
    nc = bacc.Bacc("TRN2", target_bir_lowering=False, debug=False,
                   num_devices=NCORES)
    d = {}
    for name, shape, dt_ in [
        ("wcols", [TIL, NT * JL], F32R), ("wt", [JL, IN], F32R),
        ("xtr", [TIL, NT * B], F32R),
        ("w1at", [JL, H], F32R), ("w1bt", [TIL, NT * H], F32R),
        ("w2t", [H, H], F32),
        ("w3c", [H, 1], F32), ("w1c", [H, 1], F32), ("b1c", [H, 1], F32),
        ("b2c", [H, 1], F32),
        ("b2t", [TIL, 1], F32), ("b3b", [128, 1], F32), ("biasj", [1, JL], F32R),
    ]:
        d[name] = nc.dram_tensor(name, shape, dt_, kind="ExternalInput")
    d["out"] = nc.dram_tensor("out", [128, NBT * JL], F32, kind="ExternalOutput")
    with tile.TileContext(nc) as tc:
        _build_body(nc, tc, d)
    nc.compile()
    return nc


def _perm_idx():
    """IDX[t, r] = original i for tile t, row r=12q+m -> i = 256m + 10t + q."""
    idx = np.zeros((NT, TIL), dtype=np.int64)
    valid = np.zeros((NT, TIL), dtype=bool)
    for t in range(NT):
        for q in range(10):
            for m in range(GRP):
                g = 10 * t + q
                if g < IN // GRP:
                    idx[t, 12 * q + m] = 256 * m + g
                    valid[t, 12 * q + m] = True
    return idx, valid


_IDX, _VALID = _perm_idx()


def _tiled_pad(a, axis_len=IN):
    """[3072, k] -> [120, 26*k]: row p of tile t holds a[IDX[t,p], :]."""
    k = a.shape[1]
    out = np.zeros((TIL, NT, k), dtype=np.float32)
    src = a[_IDX.reshape(-1)].reshape(NT, TIL, k)
    src[~_VALID] = 0.0
    out = src.transpose(1, 0, 2)
    return np.ascontiguousarray(out.reshape(TIL, NT * k))


def make_in_maps(X, weight, bias, W1, b1, W2, b2, W3, b3):
    f = lambda a: np.ascontiguousarray(a, dtype=np.float32)
    X, weight, bias = f(X), f(weight), f(bias)
    W1, b1, W2, b2, W3, b3 = f(W1), f(b1), f(W2), f(b2), f(W3), f(b3)
    W1aT = f(W1[:, :OUT].T)          # [3072, 10]
    W1bT = f(W1[:, OUT:OUT + IN].T)  # [3072, 10]
    xtr = _tiled_pad(f(X.T))          # [120, 26*2048]
    wT = f(weight.T)
    w1bt = _tiled_pad(W1bT)           # [120, 260]
    w2t = f(W2.T)
    w3c = f(W3.reshape(H, 1))
    w1c = f(W1[:, -1].reshape(H, 1))
    b1c, b2c = f(b1.reshape(H, 1)), f(b2.reshape(H, 1))
    b2t = f(np.tile(b2, GRP).reshape(TIL, 1))
    b3b = np.full((128, 1), b3.reshape(-1)[0], dtype=np.float32)
    in_maps = []
    for c in range(NCORES):
        j0 = c * JL
        in_maps.append({
            "wcols": _tiled_pad(f(weight[:, j0:j0 + JL])),
            "wt": f(wT[j0:j0 + JL]),
            "xtr": xtr,
            "w1at": f(W1aT[j0:j0 + JL]),
            "w1bt": w1bt,
            "w2t": w2t, "w3c": w3c, "w1c": w1c,
            "b1c": b1c, "b2c": b2c, "b2t": b2t, "b3b": b3b,
            "biasj": f(bias[j0:j0 + JL].reshape(1, JL)),
        })
    return in_maps


_NC_CACHE = {}


def kernel(X, weight, bias, W1, b1, W2, b2, W3, b3):
    global LAST_RESULTS
    if "nc" not in _NC_CACHE:
        _NC_CACHE["nc"] = build_nc()
    nc = _NC_CACHE["nc"]
    in_maps = make_in_maps(X, weight, bias, W1, b1, W2, b2, W3, b3)
    res = bass_utils.run_bass_kernel_spmd(
        nc, in_maps, core_ids=list(range(NCORES)), trace=TRACE)
    LAST_RESULTS = res
    blocks = []
    for c in range(NCORES):
        o = res.results[c]["out"]                      # [128, 16*384]
        blocks.append(o.reshape(128, NBT, JL).transpose(1, 0, 2).reshape(B, JL))
    return np.concatenate(blocks, axis=1)


# revision 14
# speedup vs baseline: 2.3545x; 1.0480x over previous
"""MetaNCA fused kernel for 8 TRN2 NeuronCores.

Sharding: the [in_units, out_units] pair grid is sharded along out_units (j)
across the 8 cores — each core owns a 384-column block of the weight matrix,
computes its block of the pairwise-MLP updates, the (local-column) bias
update, X @ new_w[:, block] and a j-sharded softmax.  Cross-core traffic is
one AllReduce of the row_part partial ([10, 3072], contracted over local j)
and two AllGathers of per-core softmax stats.

Pair-MLP mapping: groups of 12 i-rows are packed on the partition axis as
(m, h) = 12 rows x 10 hidden units = 120 partitions.  Stage 1 (relu of
row_i + col_j + b1) is a single DVE tensor_scalar per group; stages 2/3 are
PE matmuls with a block-diagonal W2^T and a shifted block-diagonal W3 that
accumulates straight into a PSUM bank pre-filled (by an identity matmul)
with the corresponding weight rows, so new_w = weight + updates materializes
in PSUM for free.

Matmuls run as float32r (single-pass fp32 on the PE; 4x the fp32 rate at
free-dim >= 256) except the weight prefill, which stays exact fp32.
Host-side input re-layout gives every big DMA >= 4KB contiguous
per-partition lines (descriptor-count, not bytes, dominates DMA time).
"""

import numpy as np

import concourse.bass as bass
import concourse.mybir as mybir
import concourse.tile as tile
from concourse import bacc
from concourse import bass_utils

F32 = mybir.dt.float32
BF16 = mybir.dt.bfloat16
F32R = mybir.dt.float32r
AX = mybir.AxisListType
OP = mybir.AluOpType
ACT = mybir.ActivationFunctionType

IN, OUT, B, H = 3072, 3072, 2048, 10
NCORES = 8
JL = OUT // NCORES          # 384 local out_units per core
GRP = 12                    # i rows packed per group (12*10 = 120 partitions)
TIL = 120                   # i rows per tile (10 groups)
NT = 26                     # ceil(3072/120); last tile has 72 valid rows
NGRP = [10] * 25 + [6]      # groups per tile
NBT = B // 128              # 16 b-tiles
HBT = NBT // 2              # 8 b-tiles per half
RG = [list(range(NCORES))]

TRACE = False               # set by test.py to capture an NTFF trace
LAST_RESULTS = None


def _build_body(nc, tc, d):
    ctx_pools = []

    def pool(name, **kw):
        cm = tc.tile_pool(name=name, **kw)
        p = cm.__enter__()
        ctx_pools.append(cm)
        return p

    p_const = pool("const", bufs=1)
    p_nw = pool("neww", bufs=1)
    p_work = pool("work", bufs=1)
    p_dram = pool("dram", bufs=1, space="DRAM")
    cm_w = tc.tile_pool(name="wcols", bufs=1)
    p_w = cm_w.__enter__()

    sync = nc.sync

    # warm up ncfw / absorb the first-collective entry barrier at t~0
    warm_in = p_dram.tile([1, 8], F32)
    warm_out = p_dram.tile([1, 8], F32)
    warm_sb = p_const.tile([1, 8], F32)
    nc.vector.memset(warm_sb[:], 1.0)
    nc.gpsimd.dma_start(warm_in[:], warm_sb[:])
    nc.gpsimd.collective_compute("AllReduce", OP.add, replica_groups=RG,
                                 ins=[warm_in.opt()], outs=[warm_out.opt()])

    # wt/w1at first: they gate a_part -> AllReduce, the critical chain
    cm_ph1 = tc.tile_pool(name="ph1", bufs=1)
    p_ph1 = cm_ph1.__enter__()
    wt_t = []
    w1at_t = []
    for c in range(3):
        wtt = p_ph1.tile([128, IN], F32R, tag=f"wt{c}", name=f"wt{c}")
        sync.dma_start(wtt[:], d["wt"].ap()[128 * c:128 * (c + 1), :])
        wt_t.append(wtt)
        w1att = p_ph1.tile([128, H], F32R, tag=f"w1at{c}", name=f"w1at{c}")
        sync.dma_start(w1att[:], d["w1at"].ap()[128 * c:128 * (c + 1), :])
        w1at_t.append(w1att)

    # ---------------- constants / small inputs -> SBUF ----------------
    w2t_sb = p_const.tile([H, H], F32)
    w3c_sb = p_const.tile([H, 1], F32)
    w1c_sb = p_const.tile([H, 1], F32)
    b1c_sb = p_const.tile([H, 1], F32)
    b2c_sb = p_const.tile([H, 1], F32)
    b2t_sb = p_const.tile([128, 1], F32)
    b3b_sb = p_const.tile([128, 1], F32)
    biasj_sb = p_const.tile([1, JL], F32R)
    for t, name in [(w2t_sb, "w2t"), (w3c_sb, "w3c"), (w1c_sb, "w1c"),
                    (b1c_sb, "b1c"), (b2c_sb, "b2c"), (b2t_sb, "b2t"),
                    (b3b_sb, "b3b"), (biasj_sb, "biasj")]:
        sync.dma_start(t[:], d[name].ap())

    ones10 = p_const.tile([1, H], F32)
    nc.vector.memset(ones10[:], 1.0)
    ones10_r = p_const.tile([1, H], F32R)
    nc.vector.tensor_scalar(ones10_r[:], ones10[:], 0.0, None, op0=OP.add)
    ones128 = p_const.tile([1, 128], F32)
    nc.vector.memset(ones128[:], 1.0)
    ones128_r = p_const.tile([1, 128], F32R)
    nc.vector.tensor_scalar(ones128_r[:], ones128[:], 0.0, None, op0=OP.add)

    # block-diag W2^T [120,120]: block m rows/cols [10m:10m+10] = W2^T
    bdw2t = p_const.tile([TIL, TIL], F32)
    nc.vector.memset(bdw2t[:], 0.0)
    # shifted block-diag W3 [120, 240]: bw3[10m+g, 108+m] = W3[0, g]
    bw3 = p_const.tile([TIL, 240], F32)
    nc.vector.memset(bw3[:], 0.0)
    for m in range(GRP):
        sync.dma_start(bdw2t[10 * m:10 * m + 10, 10 * m:10 * m + 10], w2t_sb[:])
        sync.dma_start(bw3[10 * m:10 * m + 10, 108 + m:109 + m], w3c_sb[:])
    # bf16, column/row-padded variants for the pair loop (FWL-eligible)
    bdw2t_bf = p_const.tile([TIL, 128], BF16)
    nc.vector.memset(bdw2t_bf[:], 0.0)
    nc.vector.tensor_copy(bdw2t_bf[:, :TIL], bdw2t[:])
    bw3_bf = p_const.tile([128, 248], BF16)
    nc.vector.memset(bw3_bf[:], 0.0)
    nc.vector.tensor_copy(bw3_bf[:TIL, :240], bw3[:])
    w2t_r = p_const.tile([H, H], F32R)
    nc.vector.tensor_scalar(w2t_r[:], w2t_sb[:], 0.0, None, op0=OP.add)
    w3c_r = p_const.tile([H, 1], F32R)
    nc.vector.tensor_scalar(w3c_r[:], w3c_sb[:], 0.0, None, op0=OP.add)

    # ---------------- big weight loads (re-laid-out, 1 DMA each) -------
    # wcols_all[p, t*JL + j] = weight[120t + p, j0 + j]  (zero-padded t=25)
    wcols_all = p_w.tile([TIL, NT * JL], F32R)
    sync.dma_start(wcols_all[:], d["wcols"].ap())
    wc_t = [wcols_all[:, t * JL:(t + 1) * JL] for t in range(NT)]
    # w1bt_all[p, t*H + h] = W1b[h, 120t + p]  (zero-padded)
    w1bt_all = p_const.tile([TIL, NT * H], F32R)
    sync.dma_start(w1bt_all[:], d["w1bt"].ap())
    w1bt_t = [w1bt_all[:, t * H:(t + 1) * H] for t in range(NT)]

    # ---------------- phase 1: partials + tiny prologue math ----------------
    aT_sb = p_const.tile([H, IN], F32)        # row_part^T after AllReduce
    colbT = p_const.tile([H, JL], F32)        # col_part^T + bias*W1c + b1
    tmp_sb = p_const.tile([H, JL], F32)       # bias*W1c + b1  (reused for new_b)
    cb = p_const.tile([TIL, JL], F32)         # colbT replicated 12x
    a_pk = p_const.tile([TIL, IN // GRP], F32)  # a packed [120, 256]

    with tc.tile_pool(name="ppearly", space="PSUM", bufs=1) as pp:
        psum_a = pp.tile([H, IN], F32)        # 6 banks
        psum_c = pp.tile([H, JL], F32)
        psum_bb = pp.tile([H, JL], F32)
        # row_part partial: contract over local j (k = 384 in 3 chunks)
        for s in range(IN // 512):
            for c in range(3):
                nc.tensor.matmul(psum_a[:, 512 * s:512 * (s + 1)],
                                 w1at_t[c][:],
                                 wt_t[c][:, 512 * s:512 * (s + 1)],
                                 start=(c == 0), stop=(c == 2))
        aT_part = p_ph1.tile([H, IN], BF16)
        nc.scalar.activation(aT_part[:], psum_a[:], ACT.Copy)

        # AllReduce (bf16) of row_part partial; col_part overlaps it
        ar_in = p_dram.tile([H, IN], BF16)
        ar_out = p_dram.tile([H, IN], BF16)
        nc.gpsimd.dma_start(ar_in[:], aT_part[:])
        nc.gpsimd.collective_compute("AllReduce", OP.add, replica_groups=RG,
                                     ins=[ar_in.opt()], outs=[ar_out.opt()])
        aT_bf = p_ph1.tile([H, IN], BF16)
        nc.gpsimd.dma_start(aT_bf[:], ar_out[:])
        nc.vector.tensor_copy(aT_sb[:], aT_bf[:])

        # col_part^T: contract over all i (local work, overlaps the AR)
        for t in range(NT):
            nc.tensor.matmul(psum_c[:], w1bt_t[t], wc_t[t],
                             start=(t == 0), stop=(t == NT - 1))
        # bias broadcast to 10 partitions, tmp = bias*W1c + b1
        nc.tensor.matmul(psum_bb[:], ones10_r[:], biasj_sb[:])
        nc.vector.tensor_scalar(tmp_sb[:], psum_bb[:], w1c_sb[:], b1c_sb[:],
                                op0=OP.mult, op1=OP.add)
        nc.vector.tensor_tensor(colbT[:], psum_c[:], tmp_sb[:], op=OP.add)

    cm_ph1.__exit__(None, None, None)

    # replicate colbT 12x; pack a
    for m in range(GRP):
        sync.dma_start(cb[10 * m:10 * m + 10, :], colbT[:])
        sync.dma_start(a_pk[10 * m:10 * m + 10, :], aT_sb[:, 256 * m:256 * (m + 1)])

    # ---------------- phase 2: pair-grid MLP -> new_w tiles ----------------
    nw_t = [p_nw.tile([TIL, JL], F32R, tag=f"nw{t}", name=f"nw{t}")
            for t in range(NT)]

    with tc.tile_pool(name="pph2", space="PSUM", bufs=4) as pp_h2, \
         tc.tile_pool(name="ppw", space="PSUM", bufs=2) as pp_w:
        for t in range(NT):
            ngrp = NGRP[t]
            psw = pp_w.tile([TIL, JL], F32, tag="psw", name=f"psw{t}")
            for q in range(ngrp):
                g = t * 10 + q
                h1 = p_work.tile([TIL, JL], BF16, tag="h1", name=f"h1_{g}", bufs=4)
                nc.vector.tensor_scalar(h1[:], cb[:], a_pk[:, g:g + 1],
                                        0.0, op0=OP.add, op1=OP.max)
                ph2 = pp_h2.tile([128, JL], F32, tag="ph2", name=f"ph2_{g}")
                nc.tensor.matmul(ph2[:], bdw2t_bf[:], h1[:],
                                 start=True, stop=True)
                h2 = p_work.tile([128, JL], BF16, tag="h2", name=f"h2_{g}", bufs=4)
                nc.scalar.activation(h2[:], ph2[:], ACT.Relu, bias=b2t_sb[:])
                off = 108 - GRP * q
                nc.tensor.matmul(psw[:], bw3_bf[:, off:off + TIL], h2[:],
                                 start=(q == 0), stop=(q == ngrp - 1))
            # new_w = weight + updates + b3 (exact fp32 add, rounded to f32r)
            upd = p_work.tile([TIL, JL], F32, tag="upd", name=f"upd{t}", bufs=2)
            nc.scalar.activation(upd[:], psw[:], ACT.Identity,
                                 bias=b3b_sb[:TIL, :])
            nc.vector.tensor_tensor(nw_t[t][:], upd[:], wc_t[t].bitcast(F32),
                                    op=OP.add)

    cm_w.__exit__(None, None, None)

    # ---------------- phase 3: bias update (local columns) ----------------
    nb_sb = p_const.tile([128, JL], F32)
    with tc.tile_pool(name="ppnb", space="PSUM", bufs=1) as pp_nb:
        ps_g1 = pp_nb.tile([H, JL], F32)
        for t in range(NT):
            nc.tensor.matmul(ps_g1[:], w1bt_t[t], nw_t[t][:],
                             start=(t == 0), stop=(t == NT - 1))
        g1z = p_work.tile([H, JL], F32)
        nc.vector.tensor_tensor(g1z[:], ps_g1[:], tmp_sb[:], op=OP.add)
        g1 = p_work.tile([H, JL], F32R)
        nc.vector.tensor_scalar(g1[:], g1z[:], 0.0, None, op0=OP.max)
        ps_g2 = pp_nb.tile([H, JL], F32)
        nc.tensor.matmul(ps_g2[:], w2t_r[:], g1[:], start=True, stop=True)
        g2 = p_work.tile([H, JL], F32R)
        nc.scalar.activation(g2[:], ps_g2[:], ACT.Relu, bias=b2c_sb[:])
        ps_db = pp_nb.tile([1, JL], F32)
        nc.tensor.matmul(ps_db[:], w3c_r[:], g2[:], start=True, stop=True)
        nbrow = p_work.tile([1, JL], F32)
        nc.vector.tensor_tensor(nbrow[:], ps_db[:], biasj_sb[:].bitcast(F32), op=OP.add)
        nbrow2 = p_work.tile([1, JL], F32R)
        nc.vector.tensor_scalar(nbrow2[:], nbrow[:], b3b_sb[:1, :], None, op0=OP.add)
        ps_nb = pp_nb.tile([128, JL], F32)
        nc.tensor.matmul(ps_nb[:], ones128_r[:], nbrow2[:], start=True, stop=True)
        nc.scalar.copy(nb_sb[:], ps_nb[:])

    # ------- phase 4: logits (two psum-halves) + one softmax stats exchange ---
    p_e = pool("etiles", bufs=1)
    stats = p_work.tile([128, 2 * NBT], F32)   # cols [0:16]=mx, [16:32]=s
    nmx = p_work.tile([128, NBT], F32)
    e_all = [p_e.tile([128, JL], F32, tag=f"e_{bt}", name=f"e_{bt}")
             for bt in range(NBT)]
    for half in range(2):
        with tc.tile_pool(name=f"ppl{half}", space="PSUM", bufs=1) as pp_l, \
             tc.tile_pool(name=f"xtp{half}", bufs=6) as p_xt:
            psl = [pp_l.tile([128, JL], F32, tag=f"psl{b}", name=f"psl{half}_{b}")
                   for b in range(HBT)]
            for t in range(NT):
                xt = p_xt.tile([TIL, 1024], F32R, tag="xt", name=f"xt{half}_{t}")
                off = t * B + half * 1024
                sync.dma_start(xt[:], d["xtr"].ap()[:, off:off + 1024])
                for b in range(HBT):
                    nc.tensor.matmul(psl[b][:], xt[:, 128 * b:128 * (b + 1)],
                                     nw_t[t][:],
                                     start=(t == 0), stop=(t == NT - 1))
            for b in range(HBT):
                bt = half * HBT + b
                lsb = p_work.tile([128, JL], F32, tag="lsb", name=f"lsb{bt}",
                                  bufs=2)
                nc.vector.tensor_tensor(lsb[:], psl[b][:], nb_sb[:], op=OP.add)
                nc.vector.tensor_reduce(stats[:, bt:bt + 1], lsb[:], axis=AX.X,
                                        op=OP.max)
                nc.vector.tensor_scalar(nmx[:, bt:bt + 1], stats[:, bt:bt + 1],
                                        -1.0, None, op0=OP.mult)
                nc.scalar.activation(e_all[bt][:], lsb[:], ACT.Exp,
                                     bias=nmx[:, bt:bt + 1],
                                     accum_out=stats[:, NBT + bt:NBT + bt + 1])

    mx_in = p_dram.tile([128, NBT], F32)
    mx_out = p_dram.tile([128, NBT], F32)
    sp_in = p_dram.tile([128, NBT], F32)
    sp_out = p_dram.tile([128, NBT], F32)
    nc.gpsimd.dma_start(mx_in[:], stats[:, 0:NBT])
    nc.gpsimd.collective_compute("AllReduce", OP.max, replica_groups=RG,
                                 ins=[mx_in.opt()], outs=[mx_out.opt()])
    m_gl = p_work.tile([128, NBT], F32)
    nc.gpsimd.dma_start(m_gl[:], mx_out[:])
    dif_o = p_work.tile([128, NBT], F32)
    nc.vector.tensor_tensor(dif_o[:], stats[:, 0:NBT], m_gl[:], op=OP.subtract)
    e_o = p_work.tile([128, NBT], F32)
    nc.scalar.activation(e_o[:], dif_o[:], ACT.Exp)
    sp = p_work.tile([128, NBT], F32)
    nc.vector.tensor_tensor(sp[:], e_o[:], stats[:, NBT:2 * NBT], op=OP.mult)
    nc.gpsimd.dma_start(sp_in[:], sp[:])
    nc.gpsimd.collective_compute("AllReduce", OP.add, replica_groups=RG,
                                 ins=[sp_in.opt()], outs=[sp_out.opt()])
    s_gl = p_work.tile([128, NBT], F32)
    nc.gpsimd.dma_start(s_gl[:], sp_out[:])
    rs = p_work.tile([128, NBT], F32)
    nc.vector.reciprocal(rs[:], s_gl[:])
    scl = p_work.tile([128, NBT], F32)
    nc.vector.tensor_tensor(scl[:], e_o[:], rs[:], op=OP.mult)

    oslab = p_e.tile([128, NBT * JL], F32)
    for bt in range(NBT):
        nc.vector.tensor_scalar(oslab[:, bt * JL:(bt + 1) * JL], e_all[bt][:],
                                scl[:, bt:bt + 1], None, op0=OP.mult)
    sync.dma_start(d["out"].ap(), oslab[:])

    for p in reversed(ctx_pools):
        p.__exit__(None, None, None)


def build_nc():
    nc = bacc.Bacc("TRN2", target_bir_lowering=False, debug=False,
                   num_devices=NCORES)
    d = {}
    for name, shape, dt_ in [
        ("wcols", [TIL, NT * JL], F32R), ("wt", [JL, IN], F32R),
        ("xtr", [TIL, NT * B], F32R),
        ("w1at", [JL, H], F32R), ("w1bt", [TIL, NT * H], F32R),
        ("w2t", [H, H], F32),
        ("w3c", [H, 1], F32), ("w1c", [H, 1], F32), ("b1c", [H, 1], F32),
        ("b2c", [H, 1], F32),
        ("b2t", [128, 1], F32), ("b3b", [128, 1], F32), ("biasj", [1, JL], F32R),
    ]:
        d[name] = nc.dram_tensor(name, shape, dt_, kind="ExternalInput")
    d["out"] = nc.dram_tensor("out", [128, NBT * JL], F32, kind="ExternalOutput")
    with tile.TileContext(nc) as tc:
        _build_body(nc, tc, d)
    nc.compile()
    return nc


def _perm_idx():
    """IDX[t, r] = original i for tile t, row r=12q+m -> i = 256m + 10t + q."""
    idx = np.zeros((NT, TIL), dtype=np.int64)
    valid = np.zeros((NT, TIL), dtype=bool)
    for t in range(NT):
        for q in range(10):
            for m in range(GRP):
                g = 10 * t + q
                if g < IN // GRP:
                    idx[t, 12 * q + m] = 256 * m + g
                    valid[t, 12 * q + m] = True
    return idx, valid


_IDX, _VALID = _perm_idx()


def _tiled_pad(a, axis_len=IN):
    """[3072, k] -> [120, 26*k]: row p of tile t holds a[IDX[t,p], :]."""
    k = a.shape[1]
    out = np.zeros((TIL, NT, k), dtype=np.float32)
    src = a[_IDX.reshape(-1)].reshape(NT, TIL, k)
    src[~_VALID] = 0.0
    out = src.transpose(1, 0, 2)
    return np.ascontiguousarray(out.reshape(TIL, NT * k))


def make_in_maps(X, weight, bias, W1, b1, W2, b2, W3, b3):
    f = lambda a: np.ascontiguousarray(a, dtype=np.float32)
    X, weight, bias = f(X), f(weight), f(bias)
    W1, b1, W2, b2, W3, b3 = f(W1), f(b1), f(W2), f(b2), f(W3), f(b3)
    W1aT = f(W1[:, :OUT].T)          # [3072, 10]
    W1bT = f(W1[:, OUT:OUT + IN].T)  # [3072, 10]
    xtr = _tiled_pad(f(X.T))          # [120, 26*2048]
    wT = f(weight.T)
    w1bt = _tiled_pad(W1bT)           # [120, 260]
    w2t = f(W2.T)
    w3c = f(W3.reshape(H, 1))
    w1c = f(W1[:, -1].reshape(H, 1))
    b1c, b2c = f(b1.reshape(H, 1)), f(b2.reshape(H, 1))
    b2t = f(np.concatenate([np.tile(b2, GRP), np.zeros(8)]).reshape(128, 1))
    b3b = np.full((128, 1), b3.reshape(-1)[0], dtype=np.float32)
    in_maps = []
    for c in range(NCORES):
        j0 = c * JL
        in_maps.append({
            "wcols": _tiled_pad(f(weight[:, j0:j0 + JL])),
            "wt": f(wT[j0:j0 + JL]),
            "xtr": xtr,
            "w1at": f(W1aT[j0:j0 + JL]),
            "w1bt": w1bt,
            "w2t": w2t, "w3c": w3c, "w1c": w1c,
            "b1c": b1c, "b2c": b2c, "b2t": b2t, "b3b": b3b,
            "biasj": f(bias[j0:j0 + JL].reshape(1, JL)),
        })
    return in_maps


_NC_CACHE = {}


def kernel(X, weight, bias, W1, b1, W2, b2, W3, b3):
    global LAST_RESULTS
    if "nc" not in _NC_CACHE:
        _NC_CACHE["nc"] = build_nc()
    nc = _NC_CACHE["nc"]
    in_maps = make_in_maps(X, weight, bias, W1, b1, W2, b2, W3, b3)
    res = bass_utils.run_bass_kernel_spmd(
        nc, in_maps, core_ids=list(range(NCORES)), trace=TRACE)
    LAST_RESULTS = res
    blocks = []
    for c in range(NCORES):
        o = res.results[c]["out"]                      # [128, 16*384]
        blocks.append(o.reshape(128, NBT, JL).transpose(1, 0, 2).reshape(B, JL))
    return np.concatenate(blocks, axis=1)


# revision 16
# speedup vs baseline: 2.4779x; 1.0524x over previous
"""MetaNCA fused kernel for 8 TRN2 NeuronCores.

Sharding: the [in_units, out_units] pair grid is sharded along out_units (j)
across the 8 cores — each core owns a 384-column block of the weight matrix,
computes its block of the pairwise-MLP updates, the (local-column) bias
update, X @ new_w[:, block] and a j-sharded softmax.  Cross-core traffic is
one AllReduce of the row_part partial ([10, 3072], contracted over local j)
and two AllGathers of per-core softmax stats.

Pair-MLP mapping: groups of 12 i-rows are packed on the partition axis as
(m, h) = 12 rows x 10 hidden units = 120 partitions.  Stage 1 (relu of
row_i + col_j + b1) is a single DVE tensor_scalar per group; stages 2/3 are
PE matmuls with a block-diagonal W2^T and a shifted block-diagonal W3 that
accumulates straight into a PSUM bank pre-filled (by an identity matmul)
with the corresponding weight rows, so new_w = weight + updates materializes
in PSUM for free.

Matmuls run as float32r (single-pass fp32 on the PE; 4x the fp32 rate at
free-dim >= 256) except the weight prefill, which stays exact fp32.
Host-side input re-layout gives every big DMA >= 4KB contiguous
per-partition lines (descriptor-count, not bytes, dominates DMA time).
"""

import numpy as np

import concourse.bass as bass
import concourse.mybir as mybir
import concourse.tile as tile
from concourse import bacc
from concourse import bass_utils

F32 = mybir.dt.float32
BF16 = mybir.dt.bfloat16
F32R = mybir.dt.float32r
AX = mybir.AxisListType
OP = mybir.AluOpType
ACT = mybir.ActivationFunctionType

IN, OUT, B, H = 3072, 3072, 2048, 10
NCORES = 8
JL = OUT // NCORES          # 384 local out_units per core
GRP = 12                    # i rows packed per group (12*10 = 120 partitions)
TIL = 120                   # i rows per tile (10 groups)
NT = 26                     # ceil(3072/120); last tile has 72 valid rows
NGRP = [10] * 25 + [6]      # groups per tile
NBT = B // 128              # 16 b-tiles
HBT = NBT // 2              # 8 b-tiles per half
RG = [list(range(NCORES))]

TRACE = False               # set by test.py to capture an NTFF trace
LAST_RESULTS = None


def _build_body(nc, tc, d):
    ctx_pools = []

    def pool(name, **kw):
        cm = tc.tile_pool(name=name, **kw)
        p = cm.__enter__()
        ctx_pools.append(cm)
        return p

    p_const = pool("const", bufs=1)
    p_nw = pool("neww", bufs=1)
    p_work = pool("work", bufs=1)
    p_dram = pool("dram", bufs=1, space="DRAM")
    cm_w = tc.tile_pool(name="wcols", bufs=1)
    p_w = cm_w.__enter__()

    sync = nc.sync


    # wt/w1at first: they gate a_part -> AllReduce, the critical chain
    cm_ph1 = tc.tile_pool(name="ph1", bufs=1)
    p_ph1 = cm_ph1.__enter__()
    wt_t = []
    w1at_t = []
    for c in range(3):
        wtt = p_ph1.tile([128, IN], F32R, tag=f"wt{c}", name=f"wt{c}")
        sync.dma_start(wtt[:], d["wt"].ap()[128 * c:128 * (c + 1), :])
        wt_t.append(wtt)
        w1att = p_ph1.tile([128, H], F32R, tag=f"w1at{c}", name=f"w1at{c}")
        sync.dma_start(w1att[:], d["w1at"].ap()[128 * c:128 * (c + 1), :])
        w1at_t.append(w1att)

    # ---------------- constants / small inputs -> SBUF ----------------
    w2t_sb = p_const.tile([H, H], F32)
    w3c_sb = p_const.tile([H, 1], F32)
    w1c_sb = p_const.tile([H, 1], F32)
    b1c_sb = p_const.tile([H, 1], F32)
    b2c_sb = p_const.tile([H, 1], F32)
    b2t_sb = p_const.tile([128, 1], F32)
    b3b_sb = p_const.tile([128, 1], F32)
    biasj_sb = p_const.tile([1, JL], F32R)
    for t, name in [(w2t_sb, "w2t"), (w3c_sb, "w3c"), (w1c_sb, "w1c"),
                    (b1c_sb, "b1c"), (b2c_sb, "b2c"), (b2t_sb, "b2t"),
                    (b3b_sb, "b3b"), (biasj_sb, "biasj")]:
        sync.dma_start(t[:], d[name].ap())

    ones10 = p_const.tile([1, H], F32)
    nc.vector.memset(ones10[:], 1.0)
    ones10_r = p_const.tile([1, H], F32R)
    nc.vector.tensor_scalar(ones10_r[:], ones10[:], 0.0, None, op0=OP.add)
    ones128 = p_const.tile([1, 128], F32)
    nc.vector.memset(ones128[:], 1.0)
    ones128_r = p_const.tile([1, 128], F32R)
    nc.vector.tensor_scalar(ones128_r[:], ones128[:], 0.0, None, op0=OP.add)

    # block-diag W2^T [120,120]: block m rows/cols [10m:10m+10] = W2^T
    bdw2t = p_const.tile([TIL, TIL], F32)
    nc.vector.memset(bdw2t[:], 0.0)
    # shifted block-diag W3 [120, 240]: bw3[10m+g, 108+m] = W3[0, g]
    bw3 = p_const.tile([TIL, 240], F32)
    nc.vector.memset(bw3[:], 0.0)
    for m in range(GRP):
        sync.dma_start(bdw2t[10 * m:10 * m + 10, 10 * m:10 * m + 10], w2t_sb[:])
        sync.dma_start(bw3[10 * m:10 * m + 10, 108 + m:109 + m], w3c_sb[:])
    # bf16, column/row-padded variants for the pair loop (FWL-eligible)
    bdw2t_bf = p_const.tile([TIL, 128], BF16)
    nc.vector.memset(bdw2t_bf[:], 0.0)
    nc.vector.tensor_copy(bdw2t_bf[:, :TIL], bdw2t[:])
    bw3_bf = p_const.tile([128, 248], BF16)
    nc.vector.memset(bw3_bf[:], 0.0)
    nc.vector.tensor_copy(bw3_bf[:TIL, :240], bw3[:])
    w2t_r = p_const.tile([H, H], F32R)
    nc.vector.tensor_scalar(w2t_r[:], w2t_sb[:], 0.0, None, op0=OP.add)
    w3c_r = p_const.tile([H, 1], F32R)
    nc.vector.tensor_scalar(w3c_r[:], w3c_sb[:], 0.0, None, op0=OP.add)

    # ---------------- big weight loads (re-laid-out, 1 DMA each) -------
    # wcols_all[p, t*JL + j] = weight[120t + p, j0 + j]  (zero-padded t=25)
    wcols_all = p_w.tile([TIL, NT * JL], F32R)
    sync.dma_start(wcols_all[:], d["wcols"].ap())
    wc_t = [wcols_all[:, t * JL:(t + 1) * JL] for t in range(NT)]
    # w1bt_all[p, t*H + h] = W1b[h, 120t + p]  (zero-padded)
    w1bt_all = p_const.tile([TIL, NT * H], F32R)
    sync.dma_start(w1bt_all[:], d["w1bt"].ap())
    w1bt_t = [w1bt_all[:, t * H:(t + 1) * H] for t in range(NT)]

    # ---------------- phase 1: partials + tiny prologue math ----------------
    aT_sb = p_const.tile([H, IN], F32)        # row_part^T after AllReduce
    colbT = p_const.tile([H, JL], F32)        # col_part^T + bias*W1c + b1
    tmp_sb = p_const.tile([H, JL], F32)       # bias*W1c + b1  (reused for new_b)
    cb = p_const.tile([TIL, JL], BF16)        # colbT replicated 12x (bf16)
    a_pk = p_const.tile([TIL, IN // GRP], F32)  # a packed [120, 256]

    with tc.tile_pool(name="ppearly", space="PSUM", bufs=1) as pp:
        psum_a = pp.tile([H, IN], F32)        # 6 banks
        psum_c = pp.tile([H, JL], F32)
        psum_bb = pp.tile([H, JL], F32)
        # row_part partial: contract over local j (k = 384 in 3 chunks)
        for s in range(IN // 512):
            for c in range(3):
                nc.tensor.matmul(psum_a[:, 512 * s:512 * (s + 1)],
                                 w1at_t[c][:],
                                 wt_t[c][:, 512 * s:512 * (s + 1)],
                                 start=(c == 0), stop=(c == 2))
        aT_part = p_ph1.tile([H, IN], BF16)
        nc.scalar.activation(aT_part[:], psum_a[:], ACT.Copy)

        # AllReduce (bf16) of row_part partial; col_part overlaps it
        ar_in = p_dram.tile([H, IN], BF16)
        ar_out = p_dram.tile([H, IN], BF16)
        nc.gpsimd.dma_start(ar_in[:], aT_part[:])
        nc.gpsimd.collective_compute("AllReduce", OP.add, replica_groups=RG,
                                     ins=[ar_in.opt()], outs=[ar_out.opt()])
        aT_bf = p_ph1.tile([H, IN], BF16)
        nc.gpsimd.dma_start(aT_bf[:], ar_out[:])
        nc.vector.tensor_copy(aT_sb[:], aT_bf[:])

        # col_part^T: contract over all i (local work, overlaps the AR)
        for t in range(NT):
            nc.tensor.matmul(psum_c[:], w1bt_t[t], wc_t[t],
                             start=(t == 0), stop=(t == NT - 1))
        # bias broadcast to 10 partitions, tmp = bias*W1c + b1
        nc.tensor.matmul(psum_bb[:], ones10_r[:], biasj_sb[:])
        nc.vector.tensor_scalar(tmp_sb[:], psum_bb[:], w1c_sb[:], b1c_sb[:],
                                op0=OP.mult, op1=OP.add)
        nc.vector.tensor_tensor(colbT[:], psum_c[:], tmp_sb[:], op=OP.add)

    cm_ph1.__exit__(None, None, None)

    # replicate colbT 12x (as bf16); pack a
    colbT_bf = p_const.tile([H, JL], BF16)
    nc.vector.tensor_copy(colbT_bf[:], colbT[:])
    for m in range(GRP):
        sync.dma_start(cb[10 * m:10 * m + 10, :], colbT_bf[:])
        sync.dma_start(a_pk[10 * m:10 * m + 10, :], aT_sb[:, 256 * m:256 * (m + 1)])

    # ---------------- phase 2: pair-grid MLP -> new_w tiles ----------------
    nw_t = [p_nw.tile([TIL, JL], F32R, tag=f"nw{t}", name=f"nw{t}")
            for t in range(NT)]

    with tc.tile_pool(name="pph2", space="PSUM", bufs=5) as pp_h2, \
         tc.tile_pool(name="ppw", space="PSUM", bufs=2) as pp_w:
        LA = 3
        for t in range(NT):
            ngrp = NGRP[t]
            psw = pp_w.tile([TIL, JL], F32, tag="psw", name=f"psw{t}")
            h2q = {}
            for idx in range(ngrp + LA):
                if idx < ngrp:
                    g = t * 10 + idx
                    h1 = p_work.tile([TIL, JL], BF16, tag="h1", name=f"h1_{g}",
                                     bufs=6)
                    nc.vector.tensor_scalar(h1[:], cb[:], a_pk[:, g:g + 1],
                                            0.0, op0=OP.add, op1=OP.max)
                    ph2 = pp_h2.tile([128, JL], F32, tag="ph2", name=f"ph2_{g}")
                    nc.tensor.matmul(ph2[:], bdw2t_bf[:], h1[:],
                                     start=True, stop=True)
                    h2 = p_work.tile([128, JL], BF16, tag="h2", name=f"h2_{g}",
                                     bufs=6)
                    nc.scalar.activation(h2[:], ph2[:], ACT.Relu, bias=b2t_sb[:])
                    h2q[idx] = h2
                if idx >= LA:
                    q = idx - LA
                    off = 108 - GRP * q
                    nc.tensor.matmul(psw[:], bw3_bf[:, off:off + TIL],
                                     h2q.pop(q)[:],
                                     start=(q == 0), stop=(q == ngrp - 1))
            # new_w = weight + updates + b3 (exact fp32 add, rounded to f32r)
            upd = p_work.tile([TIL, JL], F32, tag="upd", name=f"upd{t}", bufs=2)
            nc.scalar.activation(upd[:], psw[:], ACT.Identity,
                                 bias=b3b_sb[:TIL, :])
            nc.vector.tensor_tensor(nw_t[t][:], upd[:], wc_t[t].bitcast(F32),
                                    op=OP.add)

    cm_w.__exit__(None, None, None)

    # ---------------- phase 3: bias update (local columns) ----------------
    nb_sb = p_const.tile([128, JL], F32)
    with tc.tile_pool(name="ppnb", space="PSUM", bufs=1) as pp_nb:
        ps_g1 = pp_nb.tile([H, JL], F32)
        for t in range(NT):
            nc.tensor.matmul(ps_g1[:], w1bt_t[t], nw_t[t][:],
                             start=(t == 0), stop=(t == NT - 1))
        g1z = p_work.tile([H, JL], F32)
        nc.vector.tensor_tensor(g1z[:], ps_g1[:], tmp_sb[:], op=OP.add)
        g1 = p_work.tile([H, JL], F32R)
        nc.vector.tensor_scalar(g1[:], g1z[:], 0.0, None, op0=OP.max)
        ps_g2 = pp_nb.tile([H, JL], F32)
        nc.tensor.matmul(ps_g2[:], w2t_r[:], g1[:], start=True, stop=True)
        g2 = p_work.tile([H, JL], F32R)
        nc.scalar.activation(g2[:], ps_g2[:], ACT.Relu, bias=b2c_sb[:])
        ps_db = pp_nb.tile([1, JL], F32)
        nc.tensor.matmul(ps_db[:], w3c_r[:], g2[:], start=True, stop=True)
        nbrow = p_work.tile([1, JL], F32)
        nc.vector.tensor_tensor(nbrow[:], ps_db[:], biasj_sb[:].bitcast(F32), op=OP.add)
        nbrow2 = p_work.tile([1, JL], F32R)
        nc.vector.tensor_scalar(nbrow2[:], nbrow[:], b3b_sb[:1, :], None, op0=OP.add)
        ps_nb = pp_nb.tile([128, JL], F32)
        nc.tensor.matmul(ps_nb[:], ones128_r[:], nbrow2[:], start=True, stop=True)
        nc.scalar.copy(nb_sb[:], ps_nb[:])

    # ------- phase 4: logits (two psum-halves); per-half stats AR pair ---
    p_e = pool("etiles", bufs=1)
    stats = p_work.tile([128, 2 * NBT], F32)   # cols [0:16]=mx, [16:32]=s
    nmx = p_work.tile([128, NBT], F32)
    e_all = [p_e.tile([128, JL], F32, tag=f"e_{bt}", name=f"e_{bt}")
             for bt in range(NBT)]
    for half in range(2):
        with tc.tile_pool(name=f"ppl{half}", space="PSUM", bufs=1) as pp_l, \
             tc.tile_pool(name=f"xtp{half}", bufs=6) as p_xt:
            psl = [pp_l.tile([128, JL], F32, tag=f"psl{b}", name=f"psl{half}_{b}")
                   for b in range(HBT)]
            for t in range(NT - 1):
                xt = p_xt.tile([TIL, 1024], F32R, tag="xt", name=f"xt{half}_{t}")
                off = t * B + half * 1024
                sync.dma_start(xt[:], d["xtr"].ap()[:, off:off + 1024])
                for b in range(HBT):
                    nc.tensor.matmul(psl[b][:], xt[:, 128 * b:128 * (b + 1)],
                                     nw_t[t][:], start=(t == 0), stop=False)
            t = NT - 1
            xt = p_xt.tile([TIL, 1024], F32R, tag="xt", name=f"xt{half}_{t}")
            sync.dma_start(xt[:], d["xtr"].ap()[:, t * B + half * 1024:
                                                t * B + half * 1024 + 1024])
            for b in range(HBT):
                bt = half * HBT + b
                nc.tensor.matmul(psl[b][:], xt[:, 128 * b:128 * (b + 1)],
                                 nw_t[t][:], start=False, stop=True)
                lsb = p_work.tile([128, JL], F32, tag="lsb", name=f"lsb{bt}",
                                  bufs=2)
                nc.vector.tensor_tensor(lsb[:], psl[b][:], nb_sb[:], op=OP.add)
                nc.vector.tensor_reduce(stats[:, bt:bt + 1], lsb[:], axis=AX.X,
                                        op=OP.max)
                nc.vector.tensor_scalar(nmx[:, bt:bt + 1], stats[:, bt:bt + 1],
                                        -1.0, None, op0=OP.mult)
                nc.scalar.activation(e_all[bt][:], lsb[:], ACT.Exp,
                                     bias=nmx[:, bt:bt + 1],
                                     accum_out=stats[:, NBT + bt:NBT + bt + 1])

    # single exchange: T = sum_cores s_loc * e^{mx_loc}  (logits bounded ~20,
    # so e^{mx} is safely inside fp32 range for this problem size)
    emx = p_work.tile([128, NBT], F32)
    nc.scalar.activation(emx[:], stats[:, 0:NBT], ACT.Exp)
    tl = p_work.tile([128, NBT], F32)
    nc.vector.tensor_tensor(tl[:], emx[:], stats[:, NBT:2 * NBT], op=OP.mult)
    t_in = p_dram.tile([128, NBT], F32)
    t_out = p_dram.tile([128, NBT], F32)
    nc.gpsimd.dma_start(t_in[:], tl[:])
    nc.gpsimd.collective_compute("AllReduce", OP.add, replica_groups=RG,
                                 ins=[t_in.opt()], outs=[t_out.opt()])
    t_gl = p_work.tile([128, NBT], F32)
    nc.gpsimd.dma_start(t_gl[:], t_out[:])
    rs = p_work.tile([128, NBT], F32)
    nc.vector.reciprocal(rs[:], t_gl[:])
    scl = p_work.tile([128, NBT], F32)
    nc.vector.tensor_tensor(scl[:], emx[:], rs[:], op=OP.mult)
    oslab = p_e.tile([128, NBT * JL], F32)
    for bt in range(NBT):
        nc.vector.tensor_scalar(oslab[:, bt * JL:(bt + 1) * JL], e_all[bt][:],
                                scl[:, bt:bt + 1], None, op0=OP.mult)
    sync.dma_start(d["out"].ap(), oslab[:])

    for p in reversed(ctx_pools):
        p.__exit__(None, None, None)


def build_nc():
    nc = bacc.Bacc("TRN2", target_bir_lowering=False, debug=False,
                   num_devices=NCORES)
    d = {}
    for name, shape, dt_ in [
        ("wcols", [TIL, NT * JL], F32R), ("wt", [JL, IN], F32R),
        ("xtr", [TIL, NT * B], F32R),
        ("w1at", [JL, H], F32R), ("w1bt", [TIL, NT * H], F32R),
        ("w2t", [H, H], F32),
        ("w3c", [H, 1], F32), ("w1c", [H, 1], F32), ("b1c", [H, 1], F32),
        ("b2c", [H, 1], F32),
        ("b2t", [128, 1], F32), ("b3b", [128, 1], F32), ("biasj", [1, JL], F32R),
    ]:
        d[name] = nc.dram_tensor(name, shape, dt_, kind="ExternalInput")
    d["out"] = nc.dram_tensor("out", [128, NBT * JL], F32, kind="ExternalOutput")
    with tile.TileContext(nc) as tc:
        _build_body(nc, tc, d)
    nc.compile()
    return nc


def _perm_idx():
    """IDX[t, r] = original i for tile t, row r=12q+m -> i = 256m + 10t + q."""
    idx = np.zeros((NT, TIL), dtype=np.int64)
    valid = np.zeros((NT, TIL), dtype=bool)
    for t in range(NT):
        for q in range(10):
            for m in range(GRP):
                g = 10 * t + q
                if g < IN // GRP:
                    idx[t, 12 * q + m] = 256 * m + g
                    valid[t, 12 * q + m] = True
    return idx, valid


_IDX, _VALID = _perm_idx()


def _tiled_pad(a, axis_len=IN):
    """[3072, k] -> [120, 26*k]: row p of tile t holds a[IDX[t,p], :]."""
    k = a.shape[1]
    out = np.zeros((TIL, NT, k), dtype=np.float32)
    src = a[_IDX.reshape(-1)].reshape(NT, TIL, k)
    src[~_VALID] = 0.0
    out = src.transpose(1, 0, 2)
    return np.ascontiguousarray(out.reshape(TIL, NT * k))


def make_in_maps(X, weight, bias, W1, b1, W2, b2, W3, b3):
    f = lambda a: np.ascontiguousarray(a, dtype=np.float32)
    X, weight, bias = f(X), f(weight), f(bias)
    W1, b1, W2, b2, W3, b3 = f(W1), f(b1), f(W2), f(b2), f(W3), f(b3)
    W1aT = f(W1[:, :OUT].T)          # [3072, 10]
    W1bT = f(W1[:, OUT:OUT + IN].T)  # [3072, 10]
    xtr = _tiled_pad(f(X.T))          # [120, 26*2048]
    wT = f(weight.T)
    w1bt = _tiled_pad(W1bT)           # [120, 260]
    w2t = f(W2.T)
    w3c = f(W3.reshape(H, 1))
    w1c = f(W1[:, -1].reshape(H, 1))
    b1c, b2c = f(b1.reshape(H, 1)), f(b2.reshape(H, 1))
    b2t = f(np.concatenate([np.tile(b2, GRP), np.zeros(8)]).reshape(128, 1))
    b3b = np.full((128, 1), b3.reshape(-1)[0], dtype=np.float32)
    in_maps = []
    for c in range(NCORES):
        j0 = c * JL
        in_maps.append({
            "wcols": _tiled_pad(f(weight[:, j0:j0 + JL])),
            "wt": f(wT[j0:j0 + JL]),
            "xtr": xtr,
            "w1at": f(W1aT[j0:j0 + JL]),
            "w1bt": w1bt,
            "w2t": w2t, "w3c": w3c, "w1c": w1c,
            "b1c": b1c, "b2c": b2c, "b2t": b2t, "b3b": b3b,
            "biasj": f(bias[j0:j0 + JL].reshape(1, JL)),
        })
    return in_maps


_NC_CACHE = {}


def kernel(X, weight, bias, W1, b1, W2, b2, W3, b3):
    global LAST_RESULTS
    if "nc" not in _NC_CACHE:
        _NC_CACHE["nc"] = build_nc()
    nc = _NC_CACHE["nc"]
    in_maps = make_in_maps(X, weight, bias, W1, b1, W2, b2, W3, b3)
    res = bass_utils.run_bass_kernel_spmd(
        nc, in_maps, core_ids=list(range(NCORES)), trace=TRACE)
    LAST_RESULTS = res
    blocks = []
    for c in range(NCORES):
        o = res.results[c]["out"]                      # [128, 16*384]
        blocks.append(o.reshape(128, NBT, JL).transpose(1, 0, 2).reshape(B, JL))
    return np.concatenate(blocks, axis=1)
